# revision 64
# baseline (speedup 1.0000x reference)
"""Balanced E/I recurrent layer on 8 Trainium2 NeuronCores.

Contract: kernel(**inputs) takes the FULL inputs (as in reference.setup_inputs)
and returns the FULL output tuple (relu(e_T) [4096,2048] f32, ac/T, bc/T).

Strategy (data-parallel, batch 4096 -> 512 per core, SPMD on 8 cores):
  - All tensors on device are feature-major ([feature, batch_shard]).
  - State kept scaled: s = e / dt  => update s' = (1-dt)*s + net is ONE fused
    DVE op (scalar_tensor_tensor).
  - Matmul operands in fp16 (PE truncates to ~fp22 internally anyway; fp16
    runs at 1 cycle/row vs 4 for fp32), accumulation in fp32 PSUM.
  - Dale's-law scaling split to keep fp16 operands in normal range:
      weights stored as softplus(theta)/64  (negated for I-presynaptic)
      r_e stored as relu(e)/32   (64*32  = 2048 = NE fan-in)
      r_i stored as relu(i)/8    (64*8   = 512  = NI fan-in)
  - softplus computed on device as Ln(Exp(x) + 1) (no HW Softplus table).
  - act/bal cost means: per-(tile,step) partition-column sums via the ACT
    engine's accum_out, reduced on-device to 4 scalars per core; host sums
    across cores and applies the scale/normalization fixups.
"""

import numpy as np

B, D, NE, NI = 4096, 1024, 2048, 512
NCORES = 8
BS = B // NCORES          # 512 batch rows per core
N = BS                    # moving (free) dim of all matmuls
KE, KI, KD = NE // 128, NI // 128, D // 128   # 16, 4, 8 contraction tiles
ME, MI = NE // 128, NI // 128                 # 16, 4 output tiles
DT = 0.2
# Weights stored as raw softplus(theta) in fp16 (positive); the full Dale
# fan-in scale and the I-side sign live on the activations:
#   r_e16 = relu(e)/NE = Relu(s_e * DT/NE)      (ACT)
#   r_i16 = -relu(i)/NI = min(s_i * -DT/NI, 0)  (DVE)
# The tiny r16 values dip into fp16 subnormals; verified exact on the PE.
SE = DT / NE
SI_NEG = -DT / NI
# device accumulates sum(relu(s)^2) with s = e/dt  =>  relu(e)^2 = dt^2 * ...
RSQE_FIX = DT * DT
RSQI_FIX = DT * DT

_CACHE: dict[int, object] = {}
_FUSE0 = [True]  # fuse step-0 + small softplus into the drive phase


def _build_nc(T: int, bench_repeat: int = 0):
    import concourse.mybir as mybir
    import concourse.tile as tile
    from concourse import bacc

    AF = mybir.ActivationFunctionType
    f32, f16 = mybir.dt.float32, mybir.dt.float16
    ADD, MULT, MAX, MIN, BYPASS = (
        mybir.AluOpType.add, mybir.AluOpType.mult, mybir.AluOpType.max,
        mybir.AluOpType.min, mybir.AluOpType.bypass)

    # All ACT funcs used here (Exp/Ln/Identity/Relu/Square/Copy) coexist in
    # the 'natural_log_exp_and_others' table. The stock table-load pass picks
    # first-fit per func, thrashing tables (~156 loads x 1.3us). Restrict the
    # choice to the one covering table -> a single load.
    class _Bacc(bacc.Bacc):
        def insert_act_table_loads(self):
            from concourse.hw_specs import get_activation_tables
            import bass_rust as _bass_rust
            has_activation = any(
                isinstance(i, mybir.InstActivation)
                for b in self.main_func.blocks
                for i in b.instructions
            )
            if not has_activation:
                return
            tables = list(get_activation_tables(self.m.arch).items())
            full = "natural_log_exp_and_others"
            if any(n == full for n, _ in tables):
                tables = [(n, (s if n == full else set())) for n, s in tables]
            _bass_rust.insert_act_table_loads(self, tables)

    nc = _Bacc("TRN2", target_bir_lowering=False, debug=False,
               num_devices=NCORES, name="bei")

    # ---- I/O ----
    # matmul operands arrive as fp16 (host-side cast: identical rounding to
    # the on-device convert it replaces, at half the DMA bytes)
    xT = nc.dram_tensor("xT", (D, BS), f16, kind="ExternalInput").ap()
    thEET = nc.dram_tensor("thEET", (NE, NE), f16, kind="ExternalInput").ap()
    thEIT = nc.dram_tensor("thEIT", (NI, NE), f16, kind="ExternalInput").ap()
    thIET = nc.dram_tensor("thIET", (NE, NI), f16, kind="ExternalInput").ap()
    thIIT = nc.dram_tensor("thIIT", (NI, NI), f16, kind="ExternalInput").ap()
    wxeT = nc.dram_tensor("wxeT", (D, NE), f16, kind="ExternalInput").ap()
    wxiT = nc.dram_tensor("wxiT", (D, NI), f16, kind="ExternalInput").ap()
    be = nc.dram_tensor("be", (NE,), f32, kind="ExternalInput").ap()
    bi = nc.dram_tensor("bi", (NI,), f32, kind="ExternalInput").ap()
    # net-bias alone (b_e/b_i), subtracted when squaring balance = net - b
    bne = nc.dram_tensor("bne", (NE,), f32, kind="ExternalInput").ap()
    bni = nc.dram_tensor("bni", (NI,), f32, kind="ExternalInput").ap()

    e_out = nc.dram_tensor("e_out", (NE, BS), f32, kind="ExternalOutput").ap()
    sums_out = nc.dram_tensor("sums", (4, 1), f32, kind="ExternalOutput").ap()

    n_r_steps = max(T - 1, 1)  # steps contributing r^2 (j = 0..T-2)

    with tile.TileContext(nc) as tc:
        with (
            tc.tile_pool(name="persist", bufs=1) as pp,
            tc.tile_pool(name="psum", bufs=5, space="PSUM") as psp,
        ):
            # ---- persistent state ----
            s_e = pp.tile([128, KE, N], f32, tag="s_e")
            s_i = pp.tile([128, KI, N], f32, tag="s_i")
            r_e = [pp.tile([128, KE, N], f16, tag="r_e0", name="r_e0"),
                   pp.tile([128, KE, N], f16, tag="r_e1", name="r_e1")]
            r_i = pp.tile([128, KI, N], f16, tag="r_i")
            drb_e = pp.tile([128, ME, N], f16, tag="drb_e")
            drb_i = pp.tile([128, MI, N], f16, tag="drb_i")
            acc_re = pp.tile([128, n_r_steps * ME], f32, tag="acc_re")
            acc_ri = pp.tile([128, n_r_steps * MI], f32, tag="acc_ri")
            acc_be = pp.tile([128, T * ME], f32, tag="acc_be")
            acc_bi = pp.tile([128, T * MI], f32, tag="acc_bi")
            acc4 = pp.tile([128, 4], f32, tag="acc4")
            junkD = pp.tile([128, N], f32, tag="junkD")  # DVE-only discard
            sb4 = pp.tile([4, 1], f32, tag="sb4")
            be_sb = pp.tile([128, ME], f32, tag="be_sb")
            bi_sb = pp.tile([128, MI], f32, tag="bi_sb")
            nbe_sb = pp.tile([128, ME], f32, tag="nbe_sb")
            nbi_sb = pp.tile([128, MI], f32, tag="nbi_sb")

            nc.gpsimd.memset(acc_re[:], 0.0)
            nc.gpsimd.memset(acc_ri[:], 0.0)
            nc.gpsimd.memset(acc_be[:], 0.0)
            nc.gpsimd.memset(acc_bi[:], 0.0)

            nc.sync.dma_start(be_sb[:], be.rearrange("(t p) -> p t", p=128))
            nc.sync.dma_start(bi_sb[:], bi.rearrange("(t p) -> p t", p=128))
            nc.sync.dma_start(nbe_sb[:], bne.rearrange("(t p) -> p t", p=128))
            nc.sync.dma_start(nbi_sb[:], bni.rearrange("(t p) -> p t", p=128))
            nc.vector.tensor_scalar_mul(nbe_sb[:], nbe_sb[:], -1.0)
            nc.vector.tensor_scalar_mul(nbi_sb[:], nbi_sb[:], -1.0)

            # W = softplus(theta) (raw, positive, fp16), two ways split
            # across ACT and DVE:
            #   ACT path: Ln(Exp(x) + 1) -> fp16 directly
            #   DVE path: quadratic minimax fit on [-0.45, 0.45]
            #     (max err 4.8e-5, ~25x below the fp16 rounding floor)
            C0, C1, C2 = 0.69316522, 0.5, 0.12410602
            chunk_ctr = [0]
            CSZ = 512

            def softplus_chunk(theta_ap, w_tile, k, c):
                st = ts_.tile([128, CSZ], f16, tag="tchunk", name="tchunk")
                nc.sync.dma_start(
                    st[:], theta_ap[k * 128:(k + 1) * 128,
                                    c * CSZ:(c + 1) * CSZ])
                w_out = w_tile[:, k, c * CSZ:(c + 1) * CSZ]
                i = chunk_ctr[0]
                chunk_ctr[0] += 1
                if i % 8 < 3:
                    # DVE polynomial path
                    v0 = psp.tile([128, CSZ], f32, tag="pv", bufs=1,
                                  name="pv")
                    nc.vector.tensor_scalar(v0[:], st[:], C1, C0,
                                            op0=MULT, op1=ADD)
                    nc.vector.tensor_tensor(st[:], st[:], st[:], op=MULT)
                    nc.vector.scalar_tensor_tensor(w_out, st[:], C2, v0[:],
                                                   op0=MULT, op1=ADD)
                else:
                    # Exp to f32 psum scratch (avoids an extra fp16
                    # rounding between Exp and Ln)
                    ex = psp.tile([128, CSZ], f32, tag="pex", bufs=2,
                                  name="pex")
                    nc.scalar.activation(ex[:], st[:], AF.Exp)
                    nc.scalar.activation(w_out, ex[:], AF.Ln, bias=1.0)

            def softplus_into(theta_ap, w_tile, kt, width):
                for k in range(kt):
                    for c in range(width // CSZ):
                        softplus_chunk(theta_ap, w_tile, k, c)

            def emit_ee_block(b):
                # one 512-wide column block of W_EE.T = what E-groups
                # 4b..4b+3 of the next step consume
                for k in range(KE):
                    softplus_chunk(thEET, w_ee, k, b)

            # step-0 for one tile g (E tiles 0..ME-1, then I tiles):
            # s_1 = net_0 = drive(+bias); no matmuls since e_0 = 0
            def j0_tile(g, light=False):
                is_e = g < ME
                mi = g if is_e else g - ME
                drb, nb = (drb_e, nbe_sb) if is_e else (drb_i, nbi_sb)
                s_t = s_e if is_e else s_i
                nc.vector.tensor_copy(s_t[:, mi, :], drb[:, mi, :])
                if T > 1:
                    if is_e:
                        nc.scalar.activation(r_e[1][:, mi, :], s_t[:, mi, :],
                                             AF.Relu, scale=SE)
                    else:
                        nc.vector.tensor_scalar(r_i[:, mi, :], s_t[:, mi, :],
                                                SI_NEG, 0.0, op0=MULT,
                                                op1=MIN)
                if not light:
                    j0_squares(g)

            # j0's cost sums read only drb (constant), so they can run as
            # background DVE work any time later (deferred into step 1)
            def j0_squares(g):
                is_e = g < ME
                mi = g if is_e else g - ME
                drb, nb = (drb_e, nbe_sb) if is_e else (drb_i, nbi_sb)
                acc_b, acc_r = (acc_be, acc_re) if is_e else (acc_bi, acc_ri)
                # balance_0^2 = (drb - b)^2: u = drb - b, then u*u with accum
                nc.vector.tensor_scalar(junkD[:], drb[:, mi, :],
                                        nb[:, mi:mi + 1], None, op0=ADD)
                nc.vector.scalar_tensor_tensor(
                    junkD[:], junkD[:], 0.0, junkD[:], op0=BYPASS, op1=MULT,
                    accum_out=acc_b[:, mi:mi + 1])
                if T > 1:
                    # relu(s_1)^2 = max(drb,0)*drb  (s_1 == drb exactly)
                    nc.vector.scalar_tensor_tensor(
                        junkD[:], drb[:, mi, :], 0.0, drb[:, mi, :],
                        op0=MAX, op1=MULT, accum_out=acc_r[:, mi:mi + 1])

            # ---- weight pools: the small matrices coexist with the drive
            # pool (so their softplus fills ACT/DVE idle time during the
            # drive matmuls); W_EE gets the released drive space afterwards.
            theta_sched = []
            if T > 1:
                wps_cm = tc.tile_pool(name="wpool_small", bufs=1)
                wps = wps_cm.__enter__()
                w_ei = wps.tile([128, KI, NE], f16, tag="w_ei")
                w_ie = wps.tile([128, KE, NI], f16, tag="w_ie")
                w_ii = wps.tile([128, KI, NI], f16, tag="w_ii")
                ts_cm = tc.tile_pool(name="tstage", bufs=3)
                ts_ = ts_cm.__enter__()
                # priority order: what step-1's earliest groups need first
                theta_sched += [(thEIT, w_ei, k, 0) for k in range(KI)]
                theta_sched += [(thIIT, w_ii, k, 0) for k in range(KI)]
                theta_sched += [(thEIT, w_ei, k, c) for c in range(1, NE // CSZ)
                                for k in range(KI)]
                theta_sched += [(thIET, w_ie, k, 0) for k in range(KE)]
            else:
                wps_cm = ts_cm = None
            w_ee = None

            # ========= Phase 0: drive matmuls (+ fused step 0) =========
            fuse0 = T > 1 and bench_repeat == 0 and _FUSE0[0]
            with tc.tile_pool(name="dhold", bufs=1) as dh:
                x16 = dh.tile([128, KD, N], f16, tag="x16")
                wxe16 = dh.tile([128, KD, NE], f16, tag="wxe16")
                wxi16 = dh.tile([128, KD, NI], f16, tag="wxi16")

                # per-k-chunk DMAs so the first drive groups can trail the
                # transfer instead of waiting for all of it
                for k in range(KD):
                    nc.sync.dma_start(x16[:, k, :], xT[k * 128:(k + 1) * 128, :])
                    nc.sync.dma_start(wxe16[:, k, :],
                                      wxeT[k * 128:(k + 1) * 128, :])
                    nc.sync.dma_start(wxi16[:, k, :],
                                      wxiT[k * 128:(k + 1) * 128, :])

                # drive_e.T + b_e  -> drb_e (fp16), same for I
                for g in range(ME + MI):
                    is_e = g < ME
                    mi = g if is_e else g - ME
                    wx = wxe16 if is_e else wxi16
                    drb, bcol = (drb_e, be_sb) if is_e else (drb_i, bi_sb)
                    p = psp.tile([128, N], f32, tag="p")
                    for k in range(KD):
                        nc.tensor.matmul(p[:], wx[:, k, mi * 128:(mi + 1) * 128],
                                         x16[:, k, :], start=(k == 0),
                                         stop=(k == KD - 1))
                    nc.scalar.activation(drb[:, mi, :], p[:], AF.Identity,
                                         bias=bcol[:, mi:mi + 1])
                    if fuse0:
                        j0_tile(g, light=True)
                        for _ in range(3):
                            if theta_sched:
                                softplus_chunk(*theta_sched.pop(0))
            # leftover small-matrix chunks (and all of them on the
            # non-fused paths)
            while theta_sched:
                softplus_chunk(*theta_sched.pop(0))

            if T > 1:
                wee_cm = tc.tile_pool(name="wpool_ee", bufs=1)
                wee = wee_cm.__enter__()
                w_ee = wee.tile([128, KE, NE], f16, tag="w_ee")
            else:
                wee_cm = None

            # ================= Phase 2: recurrence ====================
            fo_cell = [None]
            if True:
                def emit_step(j, pre_group=None):
                    rj = r_e[j % 2]        # r_e tiles read this step
                    rn = r_e[(j + 1) % 2]  # r_e tiles written this step
                    last = (j == T - 1)

                    if j == 0:
                        for g in range(ME + MI):
                            j0_tile(g)
                        if last:
                            for mi in range(ME):
                                fo_t = fo_cell[0].tile([128, N], f32, tag="fo")
                                nc.scalar.activation(fo_t[:], s_e[:, mi, :],
                                                     AF.Relu, scale=DT)
                                nc.sync.dma_start(
                                    e_out[mi * 128:(mi + 1) * 128, :], fo_t[:])
                        return

                    # E-side groups: psum = W_EE@r_e + W_EI@r_i
                    for mi in range(ME):
                        if pre_group is not None:
                            pre_group(mi)
                        p = psp.tile([128, N], f32, tag="p")
                        for k in range(KE):
                            nc.tensor.matmul(
                                p[:], w_ee[:, k, mi * 128:(mi + 1) * 128],
                                rj[:, k, :], start=(k == 0), stop=False)
                        for k in range(KI):
                            nc.tensor.matmul(
                                p[:], w_ei[:, k, mi * 128:(mi + 1) * 128],
                                r_i[:, k, :], start=False, stop=(k == KI - 1))
                        # net = rec + drive(+b)   (in-place in PSUM)
                        nc.vector.tensor_tensor(p[:], p[:], drb_e[:, mi, :],
                                                op=ADD)
                        # s' = (1-dt)*s + net     (fused, in-place)
                        nc.vector.scalar_tensor_tensor(
                            s_e[:, mi, :], s_e[:, mi, :], 1.0 - DT, p[:],
                            op0=MULT, op1=ADD)
                        # balance^2 sums; squares scratched into dead psum
                        nc.scalar.activation(
                            p[:], p[:], AF.Square, bias=nbe_sb[:, mi:mi + 1],
                            accum_out=acc_be[:, j * ME + mi:j * ME + mi + 1])
                        if not last:
                            nc.scalar.activation(rn[:, mi, :], s_e[:, mi, :],
                                                 AF.Relu, scale=SE)
                            nc.vector.scalar_tensor_tensor(
                                junkD[:], s_e[:, mi, :], 0.0, s_e[:, mi, :],
                                op0=MAX, op1=MULT,
                                accum_out=acc_re[:, j * ME + mi:
                                                 j * ME + mi + 1])
                        else:
                            fo_t = fo_cell[0].tile([128, N], f32, tag="fo")
                            nc.scalar.activation(fo_t[:], s_e[:, mi, :],
                                                 AF.Relu, scale=DT)
                            nc.sync.dma_start(
                                e_out[mi * 128:(mi + 1) * 128, :], fo_t[:])

                    # I-side groups: psum = W_IE@r_e + W_II@r_i
                    for mi in range(MI):
                        p = psp.tile([128, N], f32, tag="p")
                        for k in range(KE):
                            nc.tensor.matmul(
                                p[:], w_ie[:, k, mi * 128:(mi + 1) * 128],
                                rj[:, k, :], start=(k == 0), stop=False)
                        for k in range(KI):
                            nc.tensor.matmul(
                                p[:], w_ii[:, k, mi * 128:(mi + 1) * 128],
                                r_i[:, k, :], start=False, stop=(k == KI - 1))
                        nc.vector.tensor_tensor(p[:], p[:], drb_i[:, mi, :],
                                                op=ADD)
                        nc.vector.scalar_tensor_tensor(
                            s_i[:, mi, :], s_i[:, mi, :], 1.0 - DT, p[:],
                            op0=MULT, op1=ADD)
                        nc.scalar.activation(
                            p[:], p[:], AF.Square, bias=nbi_sb[:, mi:mi + 1],
                            accum_out=acc_bi[:, j * MI + mi:j * MI + mi + 1])
                    # r_i is single-buffered: only overwrite it after ALL of
                    # this step's W_II matmuls (which read the old value)
                    # have been emitted. The next step's first EI matmuls
                    # wait on these, so emit them before the F squares.
                    if not last:
                        for mi in range(MI):
                            nc.vector.tensor_scalar(
                                r_i[:, mi, :], s_i[:, mi, :], SI_NEG, 0.0,
                                op0=MULT, op1=MIN)
                        for mi in range(MI):
                            nc.vector.scalar_tensor_tensor(
                                junkD[:], s_i[:, mi, :], 0.0, s_i[:, mi, :],
                                op0=MAX, op1=MULT,
                                accum_out=acc_ri[:, j * MI + mi:
                                                 j * MI + mi + 1])

                EB = NE // CSZ  # number of 512-wide W_EE column blocks (4)
                GPB = ME // EB  # E-groups consuming one block (4)

                def fout_ctx():
                    cm = tc.tile_pool(name="fout", bufs=2)
                    fo_cell[0] = cm.__enter__()
                    return cm

                if T == 1:
                    fcm = fout_ctx()
                    emit_step(0)
                    fcm.__exit__(None, None, None)
                elif bench_repeat:
                    # timing mode: repeat the matmul steps in a HW loop so
                    # device time dominates the per-launch RPC overhead
                    softplus_into(thEET, w_ee, KE, NE)
                    emit_step(0)
                    fcm = fout_ctx()
                    with tc.For_i(0, bench_repeat, 1):
                        for j in range(1, T):
                            emit_step(j)
                    fcm.__exit__(None, None, None)
                else:
                    # step 0 + small-matrix softplus already fused into the
                    # drive phase; W_EE column blocks just-in-time ahead of
                    # the step-1 groups that consume them
                    if not fuse0:
                        emit_step(0)
                    emit_ee_block(0)
                    if T >= 3:
                        def hook(mi):
                            b = mi // GPB + 1
                            if mi % GPB == 0 and b < EB:
                                emit_ee_block(b)
                            # deferred j0 cost sums: 2 tiles per E-group
                            for g in (2 * mi, 2 * mi + 1):
                                if g < ME + MI:
                                    j0_squares(g)
                        emit_step(1, pre_group=hook)
                        fcm = fout_ctx()
                        for j in range(2, T):
                            emit_step(j)
                        fcm.__exit__(None, None, None)
                    else:  # T == 2
                        for b in range(1, EB):
                            emit_ee_block(b)
                        fcm = fout_ctx()
                        emit_step(1)
                        fcm.__exit__(None, None, None)

            # ---- final scalar reduction: 4 partial sums ----
            AX = mybir.AxisListType.X
            nc.vector.reduce_sum(acc4[:, 0:1], acc_re[:], axis=AX)
            nc.vector.reduce_sum(acc4[:, 1:2], acc_ri[:], axis=AX)
            nc.vector.reduce_sum(acc4[:, 2:3], acc_be[:], axis=AX)
            nc.vector.reduce_sum(acc4[:, 3:4], acc_bi[:], axis=AX)
            ones = nc.const_aps.tensor(1.0, (128, 1), f32)
            # shares the "pv" bank (pv is prologue-only, this is end-only)
            ps4 = psp.tile([4, 1], f32, tag="pv", bufs=1, name="ps4")
            nc.tensor.matmul(ps4[:], acc4[:, 0:4], ones, start=True, stop=True)
            nc.vector.tensor_copy(sb4[:], ps4[:])
            nc.sync.dma_start(sums_out[:], sb4[:])
            if wee_cm is not None:
                wee_cm.__exit__(None, None, None)
            if ts_cm is not None:
                ts_cm.__exit__(None, None, None)
            if wps_cm is not None:
                wps_cm.__exit__(None, None, None)

    nc.finalize()
    return nc


def _get_nc(T: int):
    nc = _CACHE.get(T)
    if nc is None:
        nc = _build_nc(T)
        _CACHE[T] = nc
    return nc


def make_in_maps(inputs: dict):
    g = {k: np.ascontiguousarray(np.asarray(v, dtype=np.float32))
         for k, v in inputs.items() if k != "T"}
    f16 = np.float16
    shared = {
        "thEET": np.ascontiguousarray(g["theta_EE"].T).astype(f16),
        "thEIT": np.ascontiguousarray(g["theta_EI"].T).astype(f16),
        "thIET": np.ascontiguousarray(g["theta_IE"].T).astype(f16),
        "thIIT": np.ascontiguousarray(g["theta_II"].T).astype(f16),
        "wxeT": np.ascontiguousarray(g["W_XE_w"].T).astype(f16),
        "wxiT": np.ascontiguousarray(g["W_XI_w"].T).astype(f16),
        # drive_e + b_e is constant across steps: fold both biases into drb
        "be": (g["W_XE_b"] + g["b_e"]).astype(np.float32),
        "bi": (g["W_XI_b"] + g["b_i"]).astype(np.float32),
        # balance = net - b_e: subtract the net-bias alone when squaring
        "bne": g["b_e"],
        "bni": g["b_i"],
    }
    in_maps = []
    for c in range(NCORES):
        m = dict(shared)
        m["xT"] = np.ascontiguousarray(g["x"][c * BS:(c + 1) * BS].T).astype(f16)
        in_maps.append(m)
    return in_maps


def run(inputs: dict, trace: bool = False):
    """Run on 8 cores; returns (outputs_tuple, BassKernelResults)."""
    from concourse import bass_utils

    T = int(np.asarray(inputs["T"]))
    in_maps = make_in_maps(inputs)

    nc = _get_nc(T)
    kwargs = {}
    if trace:
        kwargs = dict(trace=True, trace_cores=[0])
    res = bass_utils.run_bass_kernel_spmd(nc, in_maps,
                                          core_ids=list(range(NCORES)),
                                          **kwargs)

    e_full = np.concatenate([res.results[c]["e_out"].T for c in range(NCORES)],
                            axis=0)
    s = np.zeros(4, dtype=np.float64)
    for c in range(NCORES):
        s += res.results[c]["sums"].astype(np.float64).ravel()
    rsqE, rsqI, bsqE, bsqI = s
    ac = (RSQE_FIX * rsqE / (B * NE) + RSQI_FIX * rsqI / (B * NI)) / T
    bc = (bsqE / (B * NE) + bsqI / (B * NI)) / T
    out = (np.ascontiguousarray(e_full, dtype=np.float32),
           np.float32(ac), np.float32(bc))
    return out, res


def kernel(**inputs):
    out, _ = run(inputs)
    return out


# revision 65
# speedup vs baseline: 238494.1101x; 238494.1101x over previous
"""Balanced E/I recurrent layer on 8 Trainium2 NeuronCores.

Contract: kernel(**inputs) takes the FULL inputs (as in reference.setup_inputs)
and returns the FULL output tuple (relu(e_T) [4096,2048] f32, ac/T, bc/T).

Strategy (data-parallel, batch 4096 -> 512 per core, SPMD on 8 cores):
  - All tensors on device are feature-major ([feature, batch_shard]); each
    recurrence step is 20 PSUM accumulation groups of 20 matmuls (N=512).
  - State kept scaled: s = e / dt  => update s' = (1-dt)*s + net is ONE fused
    DVE op (scalar_tensor_tensor); net formed in-place in PSUM.
  - Matmul operands in fp16 (PE truncates to ~fp22 internally anyway; fp16
    runs at 1 cycle/row vs 4 for fp32), accumulation in fp32 PSUM.
  - Weights stored as raw softplus(theta) fp16; the Dale fan-in scale and the
    I-presynaptic sign live on the activations (r_e = relu(e)/NE via ACT,
    r_i = -relu(i)/NI via a DVE mult+min). The tiny r values dip into fp16
    subnormals, which the PE handles exactly (verified on silicon).
  - softplus on device two ways, load-balanced across engines: ACT path
    Ln(Exp(x)+1), DVE path a quadratic fit (err 25x below the fp16 floor);
    W_EE column blocks are produced just-in-time ahead of the step-1 groups
    consuming them, and the small matrices + step 0 are fused into the
    drive-matmul phase so no engine sits behind another's queue.
  - act/bal cost sums: per-(tile,step) partition-column sums via accum_out,
    reduced on-device to 4 scalars per core (ones-matmul over partitions);
    the host sums across cores and applies the scale normalizations.
Measured: 4-step recurrence block 421.5 us on HW (263 ns/matmul; the pure
back-to-back matmul floor on this silicon measures 246.7 ns/matmul).
"""

import numpy as np

B, D, NE, NI = 4096, 1024, 2048, 512
NCORES = 8
BS = B // NCORES          # 512 batch rows per core
N = BS                    # moving (free) dim of all matmuls
KE, KI, KD = NE // 128, NI // 128, D // 128   # 16, 4, 8 contraction tiles
ME, MI = NE // 128, NI // 128                 # 16, 4 output tiles
DT = 0.2
# Weights stored as raw softplus(theta) in fp16 (positive); the full Dale
# fan-in scale and the I-side sign live on the activations:
#   r_e16 = relu(e)/NE = Relu(s_e * DT/NE)      (ACT)
#   r_i16 = -relu(i)/NI = min(s_i * -DT/NI, 0)  (DVE)
# The tiny r16 values dip into fp16 subnormals; verified exact on the PE.
SE = DT / NE
SI_NEG = -DT / NI
# device accumulates sum(relu(s)^2) with s = e/dt  =>  relu(e)^2 = dt^2 * ...
RSQE_FIX = DT * DT
RSQI_FIX = DT * DT

_CACHE: dict[int, object] = {}
_FUSE0 = [True]  # fuse step-0 + small softplus into the drive phase


def _build_nc(T: int, bench_repeat: int = 0):
    import concourse.mybir as mybir
    import concourse.tile as tile
    from concourse import bacc

    AF = mybir.ActivationFunctionType
    f32, f16 = mybir.dt.float32, mybir.dt.float16
    ADD, MULT, MAX, MIN, BYPASS = (
        mybir.AluOpType.add, mybir.AluOpType.mult, mybir.AluOpType.max,
        mybir.AluOpType.min, mybir.AluOpType.bypass)

    # All ACT funcs used here (Exp/Ln/Identity/Relu/Square/Copy) coexist in
    # the 'natural_log_exp_and_others' table. The stock table-load pass picks
    # first-fit per func, thrashing tables (~156 loads x 1.3us). Restrict the
    # choice to the one covering table -> a single load.
    class _Bacc(bacc.Bacc):
        def insert_act_table_loads(self):
            from concourse.hw_specs import get_activation_tables
            import bass_rust as _bass_rust
            has_activation = any(
                isinstance(i, mybir.InstActivation)
                for b in self.main_func.blocks
                for i in b.instructions
            )
            if not has_activation:
                return
            tables = list(get_activation_tables(self.m.arch).items())
            full = "natural_log_exp_and_others"
            if any(n == full for n, _ in tables):
                tables = [(n, (s if n == full else set())) for n, s in tables]
            _bass_rust.insert_act_table_loads(self, tables)

    nc = _Bacc("TRN2", target_bir_lowering=False, debug=False,
               num_devices=NCORES, name="bei")

    # ---- I/O ----
    # matmul operands arrive as fp16 (host-side cast: identical rounding to
    # the on-device convert it replaces, at half the DMA bytes)
    xT = nc.dram_tensor("xT", (D, BS), f16, kind="ExternalInput").ap()
    thEET = nc.dram_tensor("thEET", (NE, NE), f16, kind="ExternalInput").ap()
    thEIT = nc.dram_tensor("thEIT", (NI, NE), f16, kind="ExternalInput").ap()
    thIET = nc.dram_tensor("thIET", (NE, NI), f16, kind="ExternalInput").ap()
    thIIT = nc.dram_tensor("thIIT", (NI, NI), f16, kind="ExternalInput").ap()
    wxeT = nc.dram_tensor("wxeT", (D, NE), f16, kind="ExternalInput").ap()
    wxiT = nc.dram_tensor("wxiT", (D, NI), f16, kind="ExternalInput").ap()
    be = nc.dram_tensor("be", (NE,), f32, kind="ExternalInput").ap()
    bi = nc.dram_tensor("bi", (NI,), f32, kind="ExternalInput").ap()
    # net-bias alone (b_e/b_i), subtracted when squaring balance = net - b
    bne = nc.dram_tensor("bne", (NE,), f32, kind="ExternalInput").ap()
    bni = nc.dram_tensor("bni", (NI,), f32, kind="ExternalInput").ap()

    e_out = nc.dram_tensor("e_out", (NE, BS), f32, kind="ExternalOutput").ap()
    sums_out = nc.dram_tensor("sums", (4, 1), f32, kind="ExternalOutput").ap()

    n_r_steps = max(T - 1, 1)  # steps contributing r^2 (j = 0..T-2)

    with tile.TileContext(nc) as tc:
        with (
            tc.tile_pool(name="persist", bufs=1) as pp,
            tc.tile_pool(name="psum", bufs=5, space="PSUM") as psp,
        ):
            # ---- persistent state ----
            s_e = pp.tile([128, KE, N], f32, tag="s_e")
            s_i = pp.tile([128, KI, N], f32, tag="s_i")
            r_e = [pp.tile([128, KE, N], f16, tag="r_e0", name="r_e0"),
                   pp.tile([128, KE, N], f16, tag="r_e1", name="r_e1")]
            r_i = pp.tile([128, KI, N], f16, tag="r_i")
            drb_e = pp.tile([128, ME, N], f16, tag="drb_e")
            drb_i = pp.tile([128, MI, N], f16, tag="drb_i")
            acc_re = pp.tile([128, n_r_steps * ME], f32, tag="acc_re")
            acc_ri = pp.tile([128, n_r_steps * MI], f32, tag="acc_ri")
            acc_be = pp.tile([128, T * ME], f32, tag="acc_be")
            acc_bi = pp.tile([128, T * MI], f32, tag="acc_bi")
            acc4 = pp.tile([128, 4], f32, tag="acc4")
            junkD = pp.tile([128, N], f32, tag="junkD")  # DVE-only discard
            sb4 = pp.tile([4, 1], f32, tag="sb4")
            be_sb = pp.tile([128, ME], f32, tag="be_sb")
            bi_sb = pp.tile([128, MI], f32, tag="bi_sb")
            nbe_sb = pp.tile([128, ME], f32, tag="nbe_sb")
            nbi_sb = pp.tile([128, MI], f32, tag="nbi_sb")

            nc.gpsimd.memset(acc_re[:], 0.0)
            nc.gpsimd.memset(acc_ri[:], 0.0)
            nc.gpsimd.memset(acc_be[:], 0.0)
            nc.gpsimd.memset(acc_bi[:], 0.0)

            nc.sync.dma_start(be_sb[:], be.rearrange("(t p) -> p t", p=128))
            nc.sync.dma_start(bi_sb[:], bi.rearrange("(t p) -> p t", p=128))
            nc.sync.dma_start(nbe_sb[:], bne.rearrange("(t p) -> p t", p=128))
            nc.sync.dma_start(nbi_sb[:], bni.rearrange("(t p) -> p t", p=128))
            nc.vector.tensor_scalar_mul(nbe_sb[:], nbe_sb[:], -1.0)
            nc.vector.tensor_scalar_mul(nbi_sb[:], nbi_sb[:], -1.0)

            # W = softplus(theta) (raw, positive, fp16), two ways split
            # across ACT and DVE:
            #   ACT path: Ln(Exp(x) + 1) -> fp16 directly
            #   DVE path: quadratic minimax fit on [-0.45, 0.45]
            #     (max err 4.8e-5, ~25x below the fp16 rounding floor)
            C0, C1, C2 = 0.69316522, 0.5, 0.12410602
            chunk_ctr = [0]
            CSZ = 512

            def softplus_chunk(theta_ap, w_tile, k, c):
                st = ts_.tile([128, CSZ], f16, tag="tchunk", name="tchunk")
                nc.sync.dma_start(
                    st[:], theta_ap[k * 128:(k + 1) * 128,
                                    c * CSZ:(c + 1) * CSZ])
                w_out = w_tile[:, k, c * CSZ:(c + 1) * CSZ]
                i = chunk_ctr[0]
                chunk_ctr[0] += 1
                if i % 8 < 3:
                    # DVE polynomial path
                    v0 = psp.tile([128, CSZ], f32, tag="pv", bufs=1,
                                  name="pv")
                    nc.vector.tensor_scalar(v0[:], st[:], C1, C0,
                                            op0=MULT, op1=ADD)
                    nc.vector.tensor_tensor(st[:], st[:], st[:], op=MULT)
                    nc.vector.scalar_tensor_tensor(w_out, st[:], C2, v0[:],
                                                   op0=MULT, op1=ADD)
                else:
                    # Exp to f32 psum scratch (avoids an extra fp16
                    # rounding between Exp and Ln)
                    ex = psp.tile([128, CSZ], f32, tag="pex", bufs=2,
                                  name="pex")
                    nc.scalar.activation(ex[:], st[:], AF.Exp)
                    nc.scalar.activation(w_out, ex[:], AF.Ln, bias=1.0)

            def softplus_into(theta_ap, w_tile, kt, width):
                for k in range(kt):
                    for c in range(width // CSZ):
                        softplus_chunk(theta_ap, w_tile, k, c)

            def emit_ee_block(b):
                # one 512-wide column block of W_EE.T = what E-groups
                # 4b..4b+3 of the next step consume
                for k in range(KE):
                    softplus_chunk(thEET, w_ee, k, b)

            # step-0 for one tile g (E tiles 0..ME-1, then I tiles):
            # s_1 = net_0 = drive(+bias); no matmuls since e_0 = 0
            def j0_tile(g, light=False):
                is_e = g < ME
                mi = g if is_e else g - ME
                drb, nb = (drb_e, nbe_sb) if is_e else (drb_i, nbi_sb)
                s_t = s_e if is_e else s_i
                nc.vector.tensor_copy(s_t[:, mi, :], drb[:, mi, :])
                if T > 1:
                    if is_e:
                        nc.scalar.activation(r_e[1][:, mi, :], s_t[:, mi, :],
                                             AF.Relu, scale=SE)
                    else:
                        nc.vector.tensor_scalar(r_i[:, mi, :], s_t[:, mi, :],
                                                SI_NEG, 0.0, op0=MULT,
                                                op1=MIN)
                if not light:
                    j0_squares(g)

            # j0's cost sums read only drb (constant), so they can run as
            # background DVE work any time later (deferred into step 1)
            def j0_squares(g):
                is_e = g < ME
                mi = g if is_e else g - ME
                drb, nb = (drb_e, nbe_sb) if is_e else (drb_i, nbi_sb)
                acc_b, acc_r = (acc_be, acc_re) if is_e else (acc_bi, acc_ri)
                # balance_0^2 = (drb - b)^2: u = drb - b, then u*u with accum
                nc.vector.tensor_scalar(junkD[:], drb[:, mi, :],
                                        nb[:, mi:mi + 1], None, op0=ADD)
                nc.vector.scalar_tensor_tensor(
                    junkD[:], junkD[:], 0.0, junkD[:], op0=BYPASS, op1=MULT,
                    accum_out=acc_b[:, mi:mi + 1])
                if T > 1:
                    # relu(s_1)^2 = max(drb,0)*drb  (s_1 == drb exactly)
                    nc.vector.scalar_tensor_tensor(
                        junkD[:], drb[:, mi, :], 0.0, drb[:, mi, :],
                        op0=MAX, op1=MULT, accum_out=acc_r[:, mi:mi + 1])

            # ---- weight pools: the small matrices coexist with the drive
            # pool (so their softplus fills ACT/DVE idle time during the
            # drive matmuls); W_EE gets the released drive space afterwards.
            theta_sched = []
            if T > 1:
                wps_cm = tc.tile_pool(name="wpool_small", bufs=1)
                wps = wps_cm.__enter__()
                w_ei = wps.tile([128, KI, NE], f16, tag="w_ei")
                w_ie = wps.tile([128, KE, NI], f16, tag="w_ie")
                w_ii = wps.tile([128, KI, NI], f16, tag="w_ii")
                ts_cm = tc.tile_pool(name="tstage", bufs=3)
                ts_ = ts_cm.__enter__()
                # priority order: what step-1's earliest groups need first
                theta_sched += [(thEIT, w_ei, k, 0) for k in range(KI)]
                theta_sched += [(thIIT, w_ii, k, 0) for k in range(KI)]
                theta_sched += [(thEIT, w_ei, k, c) for c in range(1, NE // CSZ)
                                for k in range(KI)]
                theta_sched += [(thIET, w_ie, k, 0) for k in range(KE)]
            else:
                wps_cm = ts_cm = None
            w_ee = None

            # ========= Phase 0: drive matmuls (+ fused step 0) =========
            fuse0 = T > 1 and bench_repeat == 0 and _FUSE0[0]
            with tc.tile_pool(name="dhold", bufs=1) as dh:
                x16 = dh.tile([128, KD, N], f16, tag="x16")
                wxe16 = dh.tile([128, KD, NE], f16, tag="wxe16")
                wxi16 = dh.tile([128, KD, NI], f16, tag="wxi16")

                # per-k-chunk DMAs so the first drive groups can trail the
                # transfer instead of waiting for all of it
                for k in range(KD):
                    nc.sync.dma_start(x16[:, k, :], xT[k * 128:(k + 1) * 128, :])
                    nc.sync.dma_start(wxe16[:, k, :],
                                      wxeT[k * 128:(k + 1) * 128, :])
                    nc.sync.dma_start(wxi16[:, k, :],
                                      wxiT[k * 128:(k + 1) * 128, :])

                # drive_e.T + b_e  -> drb_e (fp16), same for I
                for g in range(ME + MI):
                    is_e = g < ME
                    mi = g if is_e else g - ME
                    wx = wxe16 if is_e else wxi16
                    drb, bcol = (drb_e, be_sb) if is_e else (drb_i, bi_sb)
                    p = psp.tile([128, N], f32, tag="p")
                    for k in range(KD):
                        nc.tensor.matmul(p[:], wx[:, k, mi * 128:(mi + 1) * 128],
                                         x16[:, k, :], start=(k == 0),
                                         stop=(k == KD - 1))
                    nc.scalar.activation(drb[:, mi, :], p[:], AF.Identity,
                                         bias=bcol[:, mi:mi + 1])
                    if fuse0:
                        j0_tile(g, light=True)
                        for _ in range(3):
                            if theta_sched:
                                softplus_chunk(*theta_sched.pop(0))
            # leftover small-matrix chunks (and all of them on the
            # non-fused paths)
            while theta_sched:
                softplus_chunk(*theta_sched.pop(0))

            if T > 1:
                wee_cm = tc.tile_pool(name="wpool_ee", bufs=1)
                wee = wee_cm.__enter__()
                w_ee = wee.tile([128, KE, NE], f16, tag="w_ee")
            else:
                wee_cm = None

            # ================= Phase 2: recurrence ====================
            fo_cell = [None]
            if True:
                def emit_step(j, pre_group=None):
                    rj = r_e[j % 2]        # r_e tiles read this step
                    rn = r_e[(j + 1) % 2]  # r_e tiles written this step
                    last = (j == T - 1)

                    if j == 0:
                        for g in range(ME + MI):
                            j0_tile(g)
                        if last:
                            for mi in range(ME):
                                fo_t = fo_cell[0].tile([128, N], f32, tag="fo")
                                nc.scalar.activation(fo_t[:], s_e[:, mi, :],
                                                     AF.Relu, scale=DT)
                                nc.sync.dma_start(
                                    e_out[mi * 128:(mi + 1) * 128, :], fo_t[:])
                        return

                    # E-side groups: psum = W_EE@r_e + W_EI@r_i
                    for mi in range(ME):
                        if pre_group is not None:
                            pre_group(mi)
                        p = psp.tile([128, N], f32, tag="p")
                        for k in range(KE):
                            nc.tensor.matmul(
                                p[:], w_ee[:, k, mi * 128:(mi + 1) * 128],
                                rj[:, k, :], start=(k == 0), stop=False)
                        for k in range(KI):
                            nc.tensor.matmul(
                                p[:], w_ei[:, k, mi * 128:(mi + 1) * 128],
                                r_i[:, k, :], start=False, stop=(k == KI - 1))
                        # net = rec + drive(+b)   (in-place in PSUM)
                        nc.vector.tensor_tensor(p[:], p[:], drb_e[:, mi, :],
                                                op=ADD)
                        # s' = (1-dt)*s + net     (fused, in-place)
                        nc.vector.scalar_tensor_tensor(
                            s_e[:, mi, :], s_e[:, mi, :], 1.0 - DT, p[:],
                            op0=MULT, op1=ADD)
                        # balance^2 sums; squares scratched into dead psum
                        nc.scalar.activation(
                            p[:], p[:], AF.Square, bias=nbe_sb[:, mi:mi + 1],
                            accum_out=acc_be[:, j * ME + mi:j * ME + mi + 1])
                        if not last:
                            nc.scalar.activation(rn[:, mi, :], s_e[:, mi, :],
                                                 AF.Relu, scale=SE)
                            nc.vector.scalar_tensor_tensor(
                                junkD[:], s_e[:, mi, :], 0.0, s_e[:, mi, :],
                                op0=MAX, op1=MULT,
                                accum_out=acc_re[:, j * ME + mi:
                                                 j * ME + mi + 1])
                        else:
                            fo_t = fo_cell[0].tile([128, N], f32, tag="fo")
                            nc.scalar.activation(fo_t[:], s_e[:, mi, :],
                                                 AF.Relu, scale=DT)
                            nc.sync.dma_start(
                                e_out[mi * 128:(mi + 1) * 128, :], fo_t[:])

                    # I-side groups: psum = W_IE@r_e + W_II@r_i
                    for mi in range(MI):
                        p = psp.tile([128, N], f32, tag="p")
                        for k in range(KE):
                            nc.tensor.matmul(
                                p[:], w_ie[:, k, mi * 128:(mi + 1) * 128],
                                rj[:, k, :], start=(k == 0), stop=False)
                        for k in range(KI):
                            nc.tensor.matmul(
                                p[:], w_ii[:, k, mi * 128:(mi + 1) * 128],
                                r_i[:, k, :], start=False, stop=(k == KI - 1))
                        nc.vector.tensor_tensor(p[:], p[:], drb_i[:, mi, :],
                                                op=ADD)
                        nc.vector.scalar_tensor_tensor(
                            s_i[:, mi, :], s_i[:, mi, :], 1.0 - DT, p[:],
                            op0=MULT, op1=ADD)
                        nc.scalar.activation(
                            p[:], p[:], AF.Square, bias=nbi_sb[:, mi:mi + 1],
                            accum_out=acc_bi[:, j * MI + mi:j * MI + mi + 1])
                    # r_i is single-buffered: only overwrite it after ALL of
                    # this step's W_II matmuls (which read the old value)
                    # have been emitted. The next step's first EI matmuls
                    # wait on these, so emit them before the F squares.
                    if not last:
                        for mi in range(MI):
                            nc.vector.tensor_scalar(
                                r_i[:, mi, :], s_i[:, mi, :], SI_NEG, 0.0,
                                op0=MULT, op1=MIN)
                        for mi in range(MI):
                            nc.vector.scalar_tensor_tensor(
                                junkD[:], s_i[:, mi, :], 0.0, s_i[:, mi, :],
                                op0=MAX, op1=MULT,
                                accum_out=acc_ri[:, j * MI + mi:
                                                 j * MI + mi + 1])

                EB = NE // CSZ  # number of 512-wide W_EE column blocks (4)
                GPB = ME // EB  # E-groups consuming one block (4)

                def fout_ctx():
                    cm = tc.tile_pool(name="fout", bufs=2)
                    fo_cell[0] = cm.__enter__()
                    return cm

                if T == 1:
                    fcm = fout_ctx()
                    emit_step(0)
                    fcm.__exit__(None, None, None)
                elif bench_repeat:
                    # timing mode: repeat the matmul steps in a HW loop so
                    # device time dominates the per-launch RPC overhead
                    softplus_into(thEET, w_ee, KE, NE)
                    emit_step(0)
                    fcm = fout_ctx()
                    with tc.For_i(0, bench_repeat, 1):
                        for j in range(1, T):
                            emit_step(j)
                    fcm.__exit__(None, None, None)
                else:
                    # step 0 + small-matrix softplus already fused into the
                    # drive phase; W_EE column blocks just-in-time ahead of
                    # the step-1 groups that consume them
                    if not fuse0:
                        emit_step(0)
                    emit_ee_block(0)
                    if T >= 3:
                        def hook(mi):
                            b = mi // GPB + 1
                            if mi % GPB == 0 and b < EB:
                                emit_ee_block(b)
                            # deferred j0 cost sums: 2 tiles per E-group
                            for g in (2 * mi, 2 * mi + 1):
                                if g < ME + MI:
                                    j0_squares(g)
                        emit_step(1, pre_group=hook)
                        fcm = fout_ctx()
                        for j in range(2, T):
                            emit_step(j)
                        fcm.__exit__(None, None, None)
                    else:  # T == 2
                        for b in range(1, EB):
                            emit_ee_block(b)
                        fcm = fout_ctx()
                        emit_step(1)
                        fcm.__exit__(None, None, None)

            # ---- final scalar reduction: 4 partial sums ----
            AX = mybir.AxisListType.X
            nc.vector.reduce_sum(acc4[:, 0:1], acc_re[:], axis=AX)
            nc.vector.reduce_sum(acc4[:, 1:2], acc_ri[:], axis=AX)
            nc.vector.reduce_sum(acc4[:, 2:3], acc_be[:], axis=AX)
            nc.vector.reduce_sum(acc4[:, 3:4], acc_bi[:], axis=AX)
            ones = nc.const_aps.tensor(1.0, (128, 1), f32)
            # shares the "pv" bank (pv is prologue-only, this is end-only)
            ps4 = psp.tile([4, 1], f32, tag="pv", bufs=1, name="ps4")
            nc.tensor.matmul(ps4[:], acc4[:, 0:4], ones, start=True, stop=True)
            nc.vector.tensor_copy(sb4[:], ps4[:])
            nc.sync.dma_start(sums_out[:], sb4[:])
            if wee_cm is not None:
                wee_cm.__exit__(None, None, None)
            if ts_cm is not None:
                ts_cm.__exit__(None, None, None)
            if wps_cm is not None:
                wps_cm.__exit__(None, None, None)

    nc.finalize()
    return nc


def _get_nc(T: int):
    nc = _CACHE.get(T)
    if nc is None:
        nc = _build_nc(T)
        _CACHE[T] = nc
    return nc


def make_in_maps(inputs: dict):
    g = {k: np.ascontiguousarray(np.asarray(v, dtype=np.float32))
         for k, v in inputs.items() if k != "T"}
    f16 = np.float16
    shared = {
        "thEET": np.ascontiguousarray(g["theta_EE"].T).astype(f16),
        "thEIT": np.ascontiguousarray(g["theta_EI"].T).astype(f16),
        "thIET": np.ascontiguousarray(g["theta_IE"].T).astype(f16),
        "thIIT": np.ascontiguousarray(g["theta_II"].T).astype(f16),
        "wxeT": np.ascontiguousarray(g["W_XE_w"].T).astype(f16),
        "wxiT": np.ascontiguousarray(g["W_XI_w"].T).astype(f16),
        # drive_e + b_e is constant across steps: fold both biases into drb
        "be": (g["W_XE_b"] + g["b_e"]).astype(np.float32),
        "bi": (g["W_XI_b"] + g["b_i"]).astype(np.float32),
        # balance = net - b_e: subtract the net-bias alone when squaring
        "bne": g["b_e"],
        "bni": g["b_i"],
    }
    in_maps = []
    for c in range(NCORES):
        m = dict(shared)
        m["xT"] = np.ascontiguousarray(g["x"][c * BS:(c + 1) * BS].T).astype(f16)
        in_maps.append(m)
    return in_maps


def run(inputs: dict, trace: bool = False):
    """Run on 8 cores; returns (outputs_tuple, BassKernelResults)."""
    from concourse import bass_utils

    T = int(np.asarray(inputs["T"]))
    in_maps = make_in_maps(inputs)

    nc = _get_nc(T)
    kwargs = {}
    if trace:
        kwargs = dict(trace=True, trace_cores=[0])
    res = bass_utils.run_bass_kernel_spmd(nc, in_maps,
                                          core_ids=list(range(NCORES)),
                                          **kwargs)

    e_full = np.concatenate([res.results[c]["e_out"].T for c in range(NCORES)],
                            axis=0)
    s = np.zeros(4, dtype=np.float64)
    for c in range(NCORES):
        s += res.results[c]["sums"].astype(np.float64).ravel()
    rsqE, rsqI, bsqE, bsqI = s
    ac = (RSQE_FIX * rsqE / (B * NE) + RSQI_FIX * rsqI / (B * NI)) / T
    bc = (bsqE / (B * NE) + bsqI / (B * NI)) / T
    out = (np.ascontiguousarray(e_full, dtype=np.float32),
           np.float32(ac), np.float32(bc))
    return out, res


def kernel(**inputs):
    out, _ = run(inputs)
    return out


# revision 68
# speedup vs baseline: 238846.6392x; 1.0015x over previous
"""Balanced E/I recurrent layer on 8 Trainium2 NeuronCores.

Contract: kernel(**inputs) takes the FULL inputs (as in reference.setup_inputs)
and returns the FULL output tuple (relu(e_T) [4096,2048] f32, ac/T, bc/T).

Strategy (data-parallel, batch 4096 -> 512 per core, SPMD on 8 cores):
  - All tensors on device are feature-major ([feature, batch_shard]); each
    recurrence step is 20 PSUM accumulation groups of 20 matmuls (N=512).
  - State kept scaled: s = e / dt  => update s' = (1-dt)*s + net is ONE fused
    DVE op (scalar_tensor_tensor); net formed in-place in PSUM.
  - Matmul operands in fp16 (PE truncates to ~fp22 internally anyway; fp16
    runs at 1 cycle/row vs 4 for fp32), accumulation in fp32 PSUM.
  - Weights stored as raw softplus(theta) fp16; the Dale fan-in scale and the
    I-presynaptic sign live on the activations (r_e = relu(e)/NE via ACT,
    r_i = -relu(i)/NI via a DVE mult+min). The tiny r values dip into fp16
    subnormals, which the PE handles exactly (verified on silicon).
  - softplus on device two ways, load-balanced across engines: ACT path
    Ln(Exp(x)+1), DVE path a quadratic fit (err 25x below the fp16 floor);
    W_EE column blocks are produced just-in-time ahead of the step-1 groups
    consuming them, and the small matrices + step 0 are fused into the
    drive-matmul phase so no engine sits behind another's queue.
  - act/bal cost sums: per-(tile,step) partition-column sums via accum_out,
    reduced on-device to 4 scalars per core (ones-matmul over partitions);
    the host sums across cores and applies the scale normalizations.
Measured: 4-step recurrence block 421.5 us on HW (263 ns/matmul; the pure
back-to-back matmul floor on this silicon measures 246.7 ns/matmul).
"""

import numpy as np

B, D, NE, NI = 4096, 1024, 2048, 512
NCORES = 8
BS = B // NCORES          # 512 batch rows per core
N = BS                    # moving (free) dim of all matmuls
KE, KI, KD = NE // 128, NI // 128, D // 128   # 16, 4, 8 contraction tiles
ME, MI = NE // 128, NI // 128                 # 16, 4 output tiles
DT = 0.2
# Weights stored as raw softplus(theta) in fp16 (positive); the full Dale
# fan-in scale and the I-side sign live on the activations:
#   r_e16 = relu(e)/NE = Relu(s_e * DT/NE)      (ACT)
#   r_i16 = -relu(i)/NI = min(s_i * -DT/NI, 0)  (DVE)
# The tiny r16 values dip into fp16 subnormals; verified exact on the PE.
SE = DT / NE
SI_NEG = -DT / NI
# device accumulates sum(relu(s)^2) with s = e/dt  =>  relu(e)^2 = dt^2 * ...
RSQE_FIX = DT * DT
RSQI_FIX = DT * DT

_CACHE: dict[int, object] = {}
_FUSE0 = [True]  # fuse step-0 + small softplus into the drive phase


def _build_nc(T: int, bench_repeat: int = 0):
    import concourse.mybir as mybir
    import concourse.tile as tile
    from concourse import bacc

    AF = mybir.ActivationFunctionType
    f32, f16 = mybir.dt.float32, mybir.dt.float16
    ADD, MULT, MAX, MIN, BYPASS = (
        mybir.AluOpType.add, mybir.AluOpType.mult, mybir.AluOpType.max,
        mybir.AluOpType.min, mybir.AluOpType.bypass)

    # All ACT funcs used here (Exp/Ln/Identity/Relu/Square/Copy) coexist in
    # the 'natural_log_exp_and_others' table. The stock table-load pass picks
    # first-fit per func, thrashing tables (~156 loads x 1.3us). Restrict the
    # choice to the one covering table -> a single load.
    class _Bacc(bacc.Bacc):
        def insert_act_table_loads(self):
            from concourse.hw_specs import get_activation_tables
            import bass_rust as _bass_rust
            has_activation = any(
                isinstance(i, mybir.InstActivation)
                for b in self.main_func.blocks
                for i in b.instructions
            )
            if not has_activation:
                return
            tables = list(get_activation_tables(self.m.arch).items())
            full = "natural_log_exp_and_others"
            if any(n == full for n, _ in tables):
                tables = [(n, (s if n == full else set())) for n, s in tables]
            _bass_rust.insert_act_table_loads(self, tables)

    nc = _Bacc("TRN2", target_bir_lowering=False, debug=False,
               num_devices=NCORES, name="bei")

    # ---- I/O ----
    # matmul operands arrive as fp16 (host-side cast: identical rounding to
    # the on-device convert it replaces, at half the DMA bytes)
    xT = nc.dram_tensor("xT", (D, BS), f16, kind="ExternalInput").ap()
    thEET = nc.dram_tensor("thEET", (NE, NE), f16, kind="ExternalInput").ap()
    thEIT = nc.dram_tensor("thEIT", (NI, NE), f16, kind="ExternalInput").ap()
    thIET = nc.dram_tensor("thIET", (NE, NI), f16, kind="ExternalInput").ap()
    thIIT = nc.dram_tensor("thIIT", (NI, NI), f16, kind="ExternalInput").ap()
    wxeT = nc.dram_tensor("wxeT", (D, NE), f16, kind="ExternalInput").ap()
    wxiT = nc.dram_tensor("wxiT", (D, NI), f16, kind="ExternalInput").ap()
    be = nc.dram_tensor("be", (NE,), f32, kind="ExternalInput").ap()
    bi = nc.dram_tensor("bi", (NI,), f32, kind="ExternalInput").ap()
    # net-bias alone (b_e/b_i), subtracted when squaring balance = net - b
    bne = nc.dram_tensor("bne", (NE,), f32, kind="ExternalInput").ap()
    bni = nc.dram_tensor("bni", (NI,), f32, kind="ExternalInput").ap()

    e_out = nc.dram_tensor("e_out", (NE, BS), f32, kind="ExternalOutput").ap()
    sums_out = nc.dram_tensor("sums", (4, 1), f32, kind="ExternalOutput").ap()

    n_r_steps = max(T - 1, 1)  # steps contributing r^2 (j = 0..T-2)

    with tile.TileContext(nc) as tc:
        with (
            tc.tile_pool(name="persist", bufs=1) as pp,
            tc.tile_pool(name="psum", bufs=5, space="PSUM") as psp,
        ):
            # ---- persistent state ----
            s_e = pp.tile([128, KE, N], f32, tag="s_e")
            s_i = pp.tile([128, KI, N], f32, tag="s_i")
            r_e = [pp.tile([128, KE, N], f16, tag="r_e0", name="r_e0"),
                   pp.tile([128, KE, N], f16, tag="r_e1", name="r_e1")]
            r_i = pp.tile([128, KI, N], f16, tag="r_i")
            drb_e = pp.tile([128, ME, N], f16, tag="drb_e")
            drb_i = pp.tile([128, MI, N], f16, tag="drb_i")
            acc_re = pp.tile([128, n_r_steps * ME], f32, tag="acc_re")
            acc_ri = pp.tile([128, n_r_steps * MI], f32, tag="acc_ri")
            acc_be = pp.tile([128, T * ME], f32, tag="acc_be")
            acc_bi = pp.tile([128, T * MI], f32, tag="acc_bi")
            acc4 = pp.tile([128, 4], f32, tag="acc4")
            junkD = pp.tile([128, N], f32, tag="junkD")  # DVE-only discard
            sb4 = pp.tile([4, 1], f32, tag="sb4")
            be_sb = pp.tile([128, ME], f32, tag="be_sb")
            bi_sb = pp.tile([128, MI], f32, tag="bi_sb")
            nbe_sb = pp.tile([128, ME], f32, tag="nbe_sb")
            nbi_sb = pp.tile([128, MI], f32, tag="nbi_sb")

            nc.gpsimd.memset(acc_re[:], 0.0)
            nc.gpsimd.memset(acc_ri[:], 0.0)
            nc.gpsimd.memset(acc_be[:], 0.0)
            nc.gpsimd.memset(acc_bi[:], 0.0)

            nc.sync.dma_start(be_sb[:], be.rearrange("(t p) -> p t", p=128))
            nc.sync.dma_start(bi_sb[:], bi.rearrange("(t p) -> p t", p=128))
            nc.sync.dma_start(nbe_sb[:], bne.rearrange("(t p) -> p t", p=128))
            nc.sync.dma_start(nbi_sb[:], bni.rearrange("(t p) -> p t", p=128))
            nc.vector.tensor_scalar_mul(nbe_sb[:], nbe_sb[:], -1.0)
            nc.vector.tensor_scalar_mul(nbi_sb[:], nbi_sb[:], -1.0)

            # W = softplus(theta) (raw, positive, fp16), two ways split
            # across ACT and DVE:
            #   ACT path: Ln(Exp(x) + 1) -> fp16 directly
            #   DVE path: quadratic minimax fit on [-0.45, 0.45]
            #     (max err 4.8e-5, ~25x below the fp16 rounding floor)
            C0, C1, C2 = 0.69316522, 0.5, 0.12410602
            chunk_ctr = [0]
            CSZ = 512

            def softplus_chunk(theta_ap, w_tile, k, c):
                st = ts_.tile([128, CSZ], f16, tag="tchunk", name="tchunk")
                nc.sync.dma_start(
                    st[:], theta_ap[k * 128:(k + 1) * 128,
                                    c * CSZ:(c + 1) * CSZ])
                w_out = w_tile[:, k, c * CSZ:(c + 1) * CSZ]
                i = chunk_ctr[0]
                chunk_ctr[0] += 1
                if i % 8 < 3:
                    # DVE polynomial path
                    v0 = psp.tile([128, CSZ], f32, tag="pv", bufs=1,
                                  name="pv")
                    nc.vector.tensor_scalar(v0[:], st[:], C1, C0,
                                            op0=MULT, op1=ADD)
                    nc.vector.tensor_tensor(st[:], st[:], st[:], op=MULT)
                    nc.vector.scalar_tensor_tensor(w_out, st[:], C2, v0[:],
                                                   op0=MULT, op1=ADD)
                else:
                    # Exp to f32 psum scratch (avoids an extra fp16
                    # rounding between Exp and Ln)
                    ex = psp.tile([128, CSZ], f32, tag="pex", bufs=2,
                                  name="pex")
                    nc.scalar.activation(ex[:], st[:], AF.Exp)
                    nc.scalar.activation(w_out, ex[:], AF.Ln, bias=1.0)

            def softplus_into(theta_ap, w_tile, kt, width):
                for k in range(kt):
                    for c in range(width // CSZ):
                        softplus_chunk(theta_ap, w_tile, k, c)

            def emit_ee_block(b):
                # one 512-wide column block of W_EE.T = what E-groups
                # 4b..4b+3 of the next step consume
                for k in range(KE):
                    softplus_chunk(thEET, w_ee, k, b)

            # step-0 for one tile g (E tiles 0..ME-1, then I tiles):
            # s_1 = net_0 = drive(+bias); no matmuls since e_0 = 0
            def j0_tile(g, light=False):
                is_e = g < ME
                mi = g if is_e else g - ME
                drb, nb = (drb_e, nbe_sb) if is_e else (drb_i, nbi_sb)
                s_t = s_e if is_e else s_i
                nc.vector.tensor_copy(s_t[:, mi, :], drb[:, mi, :])
                if T > 1:
                    if is_e:
                        nc.scalar.activation(r_e[1][:, mi, :], s_t[:, mi, :],
                                             AF.Relu, scale=SE)
                    else:
                        nc.vector.tensor_scalar(r_i[:, mi, :], s_t[:, mi, :],
                                                SI_NEG, 0.0, op0=MULT,
                                                op1=MIN)
                if not light:
                    j0_squares(g)

            # j0's cost sums read only drb (constant), so they can run as
            # background DVE work any time later (deferred into step 1)
            def j0_squares(g):
                is_e = g < ME
                mi = g if is_e else g - ME
                drb, nb = (drb_e, nbe_sb) if is_e else (drb_i, nbi_sb)
                acc_b, acc_r = (acc_be, acc_re) if is_e else (acc_bi, acc_ri)
                # balance_0^2 = (drb - b)^2: u = drb - b, then u*u with accum
                nc.vector.tensor_scalar(junkD[:], drb[:, mi, :],
                                        nb[:, mi:mi + 1], None, op0=ADD)
                nc.vector.scalar_tensor_tensor(
                    junkD[:], junkD[:], 0.0, junkD[:], op0=BYPASS, op1=MULT,
                    accum_out=acc_b[:, mi:mi + 1])
                if T > 1:
                    # relu(s_1)^2 = max(drb,0)*drb  (s_1 == drb exactly)
                    nc.vector.scalar_tensor_tensor(
                        junkD[:], drb[:, mi, :], 0.0, drb[:, mi, :],
                        op0=MAX, op1=MULT, accum_out=acc_r[:, mi:mi + 1])

            # ---- weight pools: the small matrices coexist with the drive
            # pool (so their softplus fills ACT/DVE idle time during the
            # drive matmuls); W_EE gets the released drive space afterwards.
            theta_sched = []
            if T > 1:
                wps_cm = tc.tile_pool(name="wpool_small", bufs=1)
                wps = wps_cm.__enter__()
                w_ei = wps.tile([128, KI, NE], f16, tag="w_ei")
                w_ie = wps.tile([128, KE, NI], f16, tag="w_ie")
                w_ii = wps.tile([128, KI, NI], f16, tag="w_ii")
                ts_cm = tc.tile_pool(name="tstage", bufs=3)
                ts_ = ts_cm.__enter__()
                # priority order: what step-1's earliest groups need first
                theta_sched += [(thEIT, w_ei, k, 0) for k in range(KI)]
                theta_sched += [(thIIT, w_ii, k, 0) for k in range(KI)]
                theta_sched += [(thEIT, w_ei, k, c) for c in range(1, NE // CSZ)
                                for k in range(KI)]
                theta_sched += [(thIET, w_ie, k, 0) for k in range(KE)]
            else:
                wps_cm = ts_cm = None
            w_ee = None

            # ========= Phase 0: drive matmuls (+ fused step 0) =========
            fuse0 = T > 1 and bench_repeat == 0 and _FUSE0[0]
            with tc.tile_pool(name="dhold", bufs=1) as dh:
                x16 = dh.tile([128, KD, N], f16, tag="x16")
                wxe16 = dh.tile([128, KD, NE], f16, tag="wxe16")
                wxi16 = dh.tile([128, KD, NI], f16, tag="wxi16")

                # per-k-chunk DMAs so the first drive groups can trail the
                # transfer instead of waiting for all of it
                for k in range(KD):
                    nc.sync.dma_start(x16[:, k, :], xT[k * 128:(k + 1) * 128, :])
                    nc.sync.dma_start(wxe16[:, k, :],
                                      wxeT[k * 128:(k + 1) * 128, :])
                    nc.sync.dma_start(wxi16[:, k, :],
                                      wxiT[k * 128:(k + 1) * 128, :])

                # drive_e.T + b_e  -> drb_e (fp16), same for I
                for g in range(ME + MI):
                    is_e = g < ME
                    mi = g if is_e else g - ME
                    wx = wxe16 if is_e else wxi16
                    drb, bcol = (drb_e, be_sb) if is_e else (drb_i, bi_sb)
                    p = psp.tile([128, N], f32, tag="p")
                    for k in range(KD):
                        nc.tensor.matmul(p[:], wx[:, k, mi * 128:(mi + 1) * 128],
                                         x16[:, k, :], start=(k == 0),
                                         stop=(k == KD - 1))
                    nc.scalar.activation(drb[:, mi, :], p[:], AF.Identity,
                                         bias=bcol[:, mi:mi + 1])
                    if fuse0:
                        j0_tile(g, light=True)
                        for _ in range(3):
                            if theta_sched:
                                softplus_chunk(*theta_sched.pop(0))
            # leftover small-matrix chunks (and all of them on the
            # non-fused paths)
            while theta_sched:
                softplus_chunk(*theta_sched.pop(0))

            if T > 1:
                wee_cm = tc.tile_pool(name="wpool_ee", bufs=1)
                wee = wee_cm.__enter__()
                w_ee = wee.tile([128, KE, NE], f16, tag="w_ee")
            else:
                wee_cm = None

            # ================= Phase 2: recurrence ====================
            fo_cell = [None]
            if True:
                def emit_step(j, pre_group=None):
                    rj = r_e[j % 2]        # r_e tiles read this step
                    rn = r_e[(j + 1) % 2]  # r_e tiles written this step
                    last = (j == T - 1)

                    if j == 0:
                        for g in range(ME + MI):
                            j0_tile(g)
                        if last:
                            for mi in range(ME):
                                fo_t = fo_cell[0].tile([128, N], f32, tag="fo")
                                nc.scalar.activation(fo_t[:], s_e[:, mi, :],
                                                     AF.Relu, scale=DT)
                                nc.sync.dma_start(
                                    e_out[mi * 128:(mi + 1) * 128, :], fo_t[:])
                        return

                    # E-side groups: psum = W_EE@r_e + W_EI@r_i
                    for mi in range(ME):
                        if pre_group is not None:
                            pre_group(mi)
                        p = psp.tile([128, N], f32, tag="p")
                        for k in range(KE):
                            nc.tensor.matmul(
                                p[:], w_ee[:, k, mi * 128:(mi + 1) * 128],
                                rj[:, k, :], start=(k == 0), stop=False)
                        for k in range(KI):
                            nc.tensor.matmul(
                                p[:], w_ei[:, k, mi * 128:(mi + 1) * 128],
                                r_i[:, k, :], start=False, stop=(k == KI - 1))
                        # net = rec + drive(+b)   (in-place in PSUM)
                        nc.vector.tensor_tensor(p[:], p[:], drb_e[:, mi, :],
                                                op=ADD)
                        # s' = (1-dt)*s + net     (fused, in-place)
                        nc.vector.scalar_tensor_tensor(
                            s_e[:, mi, :], s_e[:, mi, :], 1.0 - DT, p[:],
                            op0=MULT, op1=ADD)
                        # balance^2 sums; squares scratched into dead psum
                        nc.scalar.activation(
                            p[:], p[:], AF.Square, bias=nbe_sb[:, mi:mi + 1],
                            accum_out=acc_be[:, j * ME + mi:j * ME + mi + 1])
                        if not last:
                            nc.scalar.activation(rn[:, mi, :], s_e[:, mi, :],
                                                 AF.Relu, scale=SE)
                            nc.vector.scalar_tensor_tensor(
                                junkD[:], s_e[:, mi, :], 0.0, s_e[:, mi, :],
                                op0=MAX, op1=MULT,
                                accum_out=acc_re[:, j * ME + mi:
                                                 j * ME + mi + 1])
                        else:
                            fo_t = fo_cell[0].tile([128, N], f32, tag="fo")
                            nc.scalar.activation(fo_t[:], s_e[:, mi, :],
                                                 AF.Relu, scale=DT)
                            nc.sync.dma_start(
                                e_out[mi * 128:(mi + 1) * 128, :], fo_t[:])

                    # I-side groups: psum = W_IE@r_e + W_II@r_i
                    for mi in range(MI):
                        p = psp.tile([128, N], f32, tag="p")
                        for k in range(KE):
                            nc.tensor.matmul(
                                p[:], w_ie[:, k, mi * 128:(mi + 1) * 128],
                                rj[:, k, :], start=(k == 0), stop=False)
                        for k in range(KI):
                            nc.tensor.matmul(
                                p[:], w_ii[:, k, mi * 128:(mi + 1) * 128],
                                r_i[:, k, :], start=False, stop=(k == KI - 1))
                        nc.vector.tensor_tensor(p[:], p[:], drb_i[:, mi, :],
                                                op=ADD)
                        if not last:  # s_i after the last step is never read
                            nc.vector.scalar_tensor_tensor(
                                s_i[:, mi, :], s_i[:, mi, :], 1.0 - DT, p[:],
                                op0=MULT, op1=ADD)
                        nc.scalar.activation(
                            p[:], p[:], AF.Square, bias=nbi_sb[:, mi:mi + 1],
                            accum_out=acc_bi[:, j * MI + mi:j * MI + mi + 1])
                    # r_i is single-buffered: only overwrite it after ALL of
                    # this step's W_II matmuls (which read the old value)
                    # have been emitted. The next step's first EI matmuls
                    # wait on these, so emit them before the F squares.
                    if not last:
                        for mi in range(MI):
                            nc.vector.tensor_scalar(
                                r_i[:, mi, :], s_i[:, mi, :], SI_NEG, 0.0,
                                op0=MULT, op1=MIN)
                        for mi in range(MI):
                            nc.vector.scalar_tensor_tensor(
                                junkD[:], s_i[:, mi, :], 0.0, s_i[:, mi, :],
                                op0=MAX, op1=MULT,
                                accum_out=acc_ri[:, j * MI + mi:
                                                 j * MI + mi + 1])

                EB = NE // CSZ  # number of 512-wide W_EE column blocks (4)
                GPB = ME // EB  # E-groups consuming one block (4)

                def fout_ctx():
                    cm = tc.tile_pool(name="fout", bufs=2)
                    fo_cell[0] = cm.__enter__()
                    return cm

                if T == 1:
                    fcm = fout_ctx()
                    emit_step(0)
                    fcm.__exit__(None, None, None)
                elif bench_repeat:
                    # timing mode: repeat the matmul steps in a HW loop so
                    # device time dominates the per-launch RPC overhead
                    softplus_into(thEET, w_ee, KE, NE)
                    emit_step(0)
                    fcm = fout_ctx()
                    with tc.For_i(0, bench_repeat, 1):
                        for j in range(1, T):
                            emit_step(j)
                    fcm.__exit__(None, None, None)
                else:
                    # step 0 + small-matrix softplus already fused into the
                    # drive phase; W_EE column blocks just-in-time ahead of
                    # the step-1 groups that consume them
                    if not fuse0:
                        emit_step(0)
                    emit_ee_block(0)
                    if T >= 3:
                        def hook(mi):
                            b = mi // GPB + 1
                            if mi % GPB == 0 and b < EB:
                                emit_ee_block(b)
                            # deferred j0 cost sums: 2 tiles per E-group
                            for g in (2 * mi, 2 * mi + 1):
                                if g < ME + MI:
                                    j0_squares(g)
                        emit_step(1, pre_group=hook)
                        fcm = fout_ctx()
                        for j in range(2, T):
                            if j == T - 1:
                                # r^2 accums are complete after step T-2;
                                # reduce them in the shadow of the last step
                                AXx = mybir.AxisListType.X
                                nc.vector.reduce_sum(acc4[:, 0:1], acc_re[:],
                                                     axis=AXx)
                                nc.vector.reduce_sum(acc4[:, 1:2], acc_ri[:],
                                                     axis=AXx)
                            emit_step(j)
                        fcm.__exit__(None, None, None)
                    else:  # T == 2
                        for b in range(1, EB):
                            emit_ee_block(b)
                        fcm = fout_ctx()
                        emit_step(1)
                        fcm.__exit__(None, None, None)

            # ---- final scalar reduction: 4 partial sums ----
            AX = mybir.AxisListType.X
            if not (bench_repeat == 0 and T >= 3):
                nc.vector.reduce_sum(acc4[:, 0:1], acc_re[:], axis=AX)
                nc.vector.reduce_sum(acc4[:, 1:2], acc_ri[:], axis=AX)
            nc.vector.reduce_sum(acc4[:, 2:3], acc_be[:], axis=AX)
            nc.vector.reduce_sum(acc4[:, 3:4], acc_bi[:], axis=AX)
            ones = nc.const_aps.tensor(1.0, (128, 1), f32)
            # shares the "pv" bank (pv is prologue-only, this is end-only)
            ps4 = psp.tile([4, 1], f32, tag="pv", bufs=1, name="ps4")
            nc.tensor.matmul(ps4[:], acc4[:, 0:4], ones, start=True, stop=True)
            nc.vector.tensor_copy(sb4[:], ps4[:])
            nc.sync.dma_start(sums_out[:], sb4[:])
            if wee_cm is not None:
                wee_cm.__exit__(None, None, None)
            if ts_cm is not None:
                ts_cm.__exit__(None, None, None)
            if wps_cm is not None:
                wps_cm.__exit__(None, None, None)

    nc.finalize()
    return nc


def _get_nc(T: int):
    nc = _CACHE.get(T)
    if nc is None:
        nc = _build_nc(T)
        _CACHE[T] = nc
    return nc


def make_in_maps(inputs: dict):
    g = {k: np.ascontiguousarray(np.asarray(v, dtype=np.float32))
         for k, v in inputs.items() if k != "T"}
    f16 = np.float16
    shared = {
        "thEET": np.ascontiguousarray(g["theta_EE"].T).astype(f16),
        "thEIT": np.ascontiguousarray(g["theta_EI"].T).astype(f16),
        "thIET": np.ascontiguousarray(g["theta_IE"].T).astype(f16),
        "thIIT": np.ascontiguousarray(g["theta_II"].T).astype(f16),
        "wxeT": np.ascontiguousarray(g["W_XE_w"].T).astype(f16),
        "wxiT": np.ascontiguousarray(g["W_XI_w"].T).astype(f16),
        # drive_e + b_e is constant across steps: fold both biases into drb
        "be": (g["W_XE_b"] + g["b_e"]).astype(np.float32),
        "bi": (g["W_XI_b"] + g["b_i"]).astype(np.float32),
        # balance = net - b_e: subtract the net-bias alone when squaring
        "bne": g["b_e"],
        "bni": g["b_i"],
    }
    in_maps = []
    for c in range(NCORES):
        m = dict(shared)
        m["xT"] = np.ascontiguousarray(g["x"][c * BS:(c + 1) * BS].T).astype(f16)
        in_maps.append(m)
    return in_maps


def run(inputs: dict, trace: bool = False):
    """Run on 8 cores; returns (outputs_tuple, BassKernelResults)."""
    from concourse import bass_utils

    T = int(np.asarray(inputs["T"]))
    in_maps = make_in_maps(inputs)

    nc = _get_nc(T)
    kwargs = {}
    if trace:
        kwargs = dict(trace=True, trace_cores=[0])
    res = bass_utils.run_bass_kernel_spmd(nc, in_maps,
                                          core_ids=list(range(NCORES)),
                                          **kwargs)

    e_full = np.concatenate([res.results[c]["e_out"].T for c in range(NCORES)],
                            axis=0)
    s = np.zeros(4, dtype=np.float64)
    for c in range(NCORES):
        s += res.results[c]["sums"].astype(np.float64).ravel()
    rsqE, rsqI, bsqE, bsqI = s
    ac = (RSQE_FIX * rsqE / (B * NE) + RSQI_FIX * rsqI / (B * NI)) / T
    bc = (bsqE / (B * NE) + bsqI / (B * NI)) / T
    out = (np.ascontiguousarray(e_full, dtype=np.float32),
           np.float32(ac), np.float32(bc))
    return out, res


def kernel(**inputs):
    out, _ = run(inputs)
    return out


# revision 70
# speedup vs baseline: 246072.8011x; 1.0303x over previous
"""Balanced E/I recurrent layer on 8 Trainium2 NeuronCores.

Contract: kernel(**inputs) takes the FULL inputs (as in reference.setup_inputs)
and returns the FULL output tuple (relu(e_T) [4096,2048] f32, ac/T, bc/T).

Strategy (data-parallel, batch 4096 -> 512 per core, SPMD on 8 cores):
  - All tensors on device are feature-major ([feature, batch_shard]); each
    recurrence step is 20 PSUM accumulation groups of 20 matmuls (N=512).
  - State kept scaled: s = e / dt  => update s' = (1-dt)*s + net is ONE fused
    DVE op (scalar_tensor_tensor); net formed in-place in PSUM.
  - Matmul operands in fp16 (PE truncates to ~fp22 internally anyway; fp16
    runs at 1 cycle/row vs 4 for fp32), accumulation in fp32 PSUM.
  - Weights stored as raw softplus(theta) fp16; the Dale fan-in scale and the
    I-presynaptic sign live on the activations (r_e = relu(e)/NE via ACT,
    r_i = -relu(i)/NI via a DVE mult+min). The tiny r values dip into fp16
    subnormals, which the PE handles exactly (verified on silicon).
  - softplus on device two ways, load-balanced across engines: ACT path
    Ln(Exp(x)+1), DVE path a quadratic fit (err 25x below the fp16 floor);
    W_EE column blocks are produced just-in-time ahead of the step-1 groups
    consuming them, and the small matrices + step 0 are fused into the
    drive-matmul phase so no engine sits behind another's queue.
  - act/bal cost sums: per-(tile,step) partition-column sums via accum_out,
    reduced on-device to 4 scalars per core (ones-matmul over partitions);
    the host sums across cores and applies the scale normalizations.
Measured (For_i repeat-slope on silicon): 4-step recurrence block ~405 us
(~253 ns/matmul vs a pure back-to-back matmul floor of 246.7 ns/matmul on
this silicon, i.e. ~97% of the achievable PE rate).
"""

import numpy as np

B, D, NE, NI = 4096, 1024, 2048, 512
NCORES = 8
BS = B // NCORES          # 512 batch rows per core
N = BS                    # moving (free) dim of all matmuls
KE, KI, KD = NE // 128, NI // 128, D // 128   # 16, 4, 8 contraction tiles
ME, MI = NE // 128, NI // 128                 # 16, 4 output tiles
DT = 0.2
# Weights stored as raw softplus(theta) in fp16 (positive); the full Dale
# fan-in scale and the I-side sign live on the activations:
#   r_e16 = relu(e)/NE = Relu(s_e * DT/NE)      (ACT)
#   r_i16 = -relu(i)/NI = min(s_i * -DT/NI, 0)  (DVE)
# The tiny r16 values dip into fp16 subnormals; verified exact on the PE.
SE = DT / NE
SI_NEG = -DT / NI
# device accumulates sum(relu(s)^2) with s = e/dt  =>  relu(e)^2 = dt^2 * ...
RSQE_FIX = DT * DT
RSQI_FIX = DT * DT

_CACHE: dict[int, object] = {}
_FUSE0 = [True]  # fuse step-0 + small softplus into the drive phase


def _build_nc(T: int, bench_repeat: int = 0):
    import concourse.mybir as mybir
    import concourse.tile as tile
    from concourse import bacc

    AF = mybir.ActivationFunctionType
    f32, f16 = mybir.dt.float32, mybir.dt.float16
    ADD, MULT, MAX, MIN, BYPASS = (
        mybir.AluOpType.add, mybir.AluOpType.mult, mybir.AluOpType.max,
        mybir.AluOpType.min, mybir.AluOpType.bypass)

    # All ACT funcs used here (Exp/Ln/Identity/Relu/Square/Copy) coexist in
    # the 'natural_log_exp_and_others' table. The stock table-load pass picks
    # first-fit per func, thrashing tables (~156 loads x 1.3us). Restrict the
    # choice to the one covering table -> a single load.
    class _Bacc(bacc.Bacc):
        def insert_act_table_loads(self):
            from concourse.hw_specs import get_activation_tables
            import bass_rust as _bass_rust
            has_activation = any(
                isinstance(i, mybir.InstActivation)
                for b in self.main_func.blocks
                for i in b.instructions
            )
            if not has_activation:
                return
            tables = list(get_activation_tables(self.m.arch).items())
            full = "natural_log_exp_and_others"
            if any(n == full for n, _ in tables):
                tables = [(n, (s if n == full else set())) for n, s in tables]
            _bass_rust.insert_act_table_loads(self, tables)

    nc = _Bacc("TRN2", target_bir_lowering=False, debug=False,
               num_devices=NCORES, name="bei")

    # ---- I/O ----
    # matmul operands arrive as fp16 (host-side cast: identical rounding to
    # the on-device convert it replaces, at half the DMA bytes)
    xT = nc.dram_tensor("xT", (D, BS), f16, kind="ExternalInput").ap()
    thEET = nc.dram_tensor("thEET", (NE, NE), f16, kind="ExternalInput").ap()
    thEIT = nc.dram_tensor("thEIT", (NI, NE), f16, kind="ExternalInput").ap()
    thIET = nc.dram_tensor("thIET", (NE, NI), f16, kind="ExternalInput").ap()
    thIIT = nc.dram_tensor("thIIT", (NI, NI), f16, kind="ExternalInput").ap()
    wxeT = nc.dram_tensor("wxeT", (D, NE), f16, kind="ExternalInput").ap()
    wxiT = nc.dram_tensor("wxiT", (D, NI), f16, kind="ExternalInput").ap()
    be = nc.dram_tensor("be", (NE,), f32, kind="ExternalInput").ap()
    bi = nc.dram_tensor("bi", (NI,), f32, kind="ExternalInput").ap()
    # net-bias alone (b_e/b_i), subtracted when squaring balance = net - b
    bne = nc.dram_tensor("bne", (NE,), f32, kind="ExternalInput").ap()
    bni = nc.dram_tensor("bni", (NI,), f32, kind="ExternalInput").ap()

    e_out = nc.dram_tensor("e_out", (NE, BS), f32, kind="ExternalOutput").ap()
    sums_out = nc.dram_tensor("sums", (4, 1), f32, kind="ExternalOutput").ap()

    n_r_steps = max(T - 1, 1)  # steps contributing r^2 (j = 0..T-2)

    with tile.TileContext(nc) as tc:
        with (
            tc.tile_pool(name="persist", bufs=1) as pp,
            tc.tile_pool(name="psum", bufs=5, space="PSUM") as psp,
        ):
            # ---- persistent state ----
            s_e = pp.tile([128, KE, N], f32, tag="s_e")
            s_i = pp.tile([128, KI, N], f32, tag="s_i")
            r_e = [pp.tile([128, KE, N], f16, tag="r_e0", name="r_e0"),
                   pp.tile([128, KE, N], f16, tag="r_e1", name="r_e1")]
            r_i = pp.tile([128, KI, N], f16, tag="r_i")
            drb_e = pp.tile([128, ME, N], f16, tag="drb_e")
            drb_i = pp.tile([128, MI, N], f16, tag="drb_i")
            acc_re = pp.tile([128, n_r_steps * ME], f32, tag="acc_re")
            acc_ri = pp.tile([128, n_r_steps * MI], f32, tag="acc_ri")
            acc_be = pp.tile([128, T * ME], f32, tag="acc_be")
            acc_bi = pp.tile([128, T * MI], f32, tag="acc_bi")
            acc4 = pp.tile([128, 4], f32, tag="acc4")
            junkD = pp.tile([128, N], f32, tag="junkD")  # DVE-only discard
            sb4 = pp.tile([4, 1], f32, tag="sb4")
            be_sb = pp.tile([128, ME], f32, tag="be_sb")
            bi_sb = pp.tile([128, MI], f32, tag="bi_sb")
            nbe_sb = pp.tile([128, ME], f32, tag="nbe_sb")
            nbi_sb = pp.tile([128, MI], f32, tag="nbi_sb")

            nc.gpsimd.memset(acc_re[:], 0.0)
            nc.gpsimd.memset(acc_ri[:], 0.0)
            nc.gpsimd.memset(acc_be[:], 0.0)
            nc.gpsimd.memset(acc_bi[:], 0.0)

            nc.sync.dma_start(be_sb[:], be.rearrange("(t p) -> p t", p=128))
            nc.sync.dma_start(bi_sb[:], bi.rearrange("(t p) -> p t", p=128))
            nc.sync.dma_start(nbe_sb[:], bne.rearrange("(t p) -> p t", p=128))
            nc.sync.dma_start(nbi_sb[:], bni.rearrange("(t p) -> p t", p=128))
            nc.vector.tensor_scalar_mul(nbe_sb[:], nbe_sb[:], -1.0)
            nc.vector.tensor_scalar_mul(nbi_sb[:], nbi_sb[:], -1.0)

            # W = softplus(theta) (raw, positive, fp16), two ways split
            # across ACT and DVE:
            #   ACT path: Ln(Exp(x) + 1) -> fp16 directly
            #   DVE path: quadratic minimax fit on [-0.45, 0.45]
            #     (max err 4.8e-5, ~25x below the fp16 rounding floor)
            C0, C1, C2 = 0.69316522, 0.5, 0.12410602
            chunk_ctr = [0]
            CSZ = 512

            def softplus_chunk(theta_ap, w_tile, k, c):
                st = ts_.tile([128, CSZ], f16, tag="tchunk", name="tchunk")
                nc.sync.dma_start(
                    st[:], theta_ap[k * 128:(k + 1) * 128,
                                    c * CSZ:(c + 1) * CSZ])
                w_out = w_tile[:, k, c * CSZ:(c + 1) * CSZ]
                i = chunk_ctr[0]
                chunk_ctr[0] += 1
                if i % 8 < 3:
                    # DVE polynomial path
                    v0 = psp.tile([128, CSZ], f32, tag="pv", bufs=1,
                                  name="pv")
                    nc.vector.tensor_scalar(v0[:], st[:], C1, C0,
                                            op0=MULT, op1=ADD)
                    nc.vector.tensor_tensor(st[:], st[:], st[:], op=MULT)
                    nc.vector.scalar_tensor_tensor(w_out, st[:], C2, v0[:],
                                                   op0=MULT, op1=ADD)
                else:
                    # Exp to f32 psum scratch (avoids an extra fp16
                    # rounding between Exp and Ln)
                    ex = psp.tile([128, CSZ], f32, tag="pex", bufs=2,
                                  name="pex")
                    nc.scalar.activation(ex[:], st[:], AF.Exp)
                    nc.scalar.activation(w_out, ex[:], AF.Ln, bias=1.0)

            def softplus_into(theta_ap, w_tile, kt, width):
                for k in range(kt):
                    for c in range(width // CSZ):
                        softplus_chunk(theta_ap, w_tile, k, c)

            def emit_ee_block(b):
                # one 512-wide column block of W_EE.T = what E-groups
                # 4b..4b+3 of the next step consume
                for k in range(KE):
                    softplus_chunk(thEET, w_ee, k, b)

            # step-0 for one tile g (E tiles 0..ME-1, then I tiles):
            # s_1 = net_0 = drive(+bias); no matmuls since e_0 = 0
            def j0_tile(g, light=False):
                is_e = g < ME
                mi = g if is_e else g - ME
                drb, nb = (drb_e, nbe_sb) if is_e else (drb_i, nbi_sb)
                s_t = s_e if is_e else s_i
                nc.vector.tensor_copy(s_t[:, mi, :], drb[:, mi, :])
                if T > 1:
                    if is_e:
                        # on DVE (not ACT) to keep the drive window balanced:
                        # (s*SE) max 0 == relu(s*SE) bit-for-bit
                        nc.vector.tensor_scalar(r_e[1][:, mi, :],
                                                s_t[:, mi, :], SE, 0.0,
                                                op0=MULT, op1=MAX)
                    else:
                        nc.vector.tensor_scalar(r_i[:, mi, :], s_t[:, mi, :],
                                                SI_NEG, 0.0, op0=MULT,
                                                op1=MIN)
                if not light:
                    j0_squares(g)

            # j0's cost sums read only drb (constant), so they can run as
            # background DVE work any time later (deferred into step 1)
            def j0_squares(g):
                is_e = g < ME
                mi = g if is_e else g - ME
                drb, nb = (drb_e, nbe_sb) if is_e else (drb_i, nbi_sb)
                acc_b, acc_r = (acc_be, acc_re) if is_e else (acc_bi, acc_ri)
                # balance_0^2 = (drb - b)^2: u = drb - b, then u*u with accum
                nc.vector.tensor_scalar(junkD[:], drb[:, mi, :],
                                        nb[:, mi:mi + 1], None, op0=ADD)
                nc.vector.scalar_tensor_tensor(
                    junkD[:], junkD[:], 0.0, junkD[:], op0=BYPASS, op1=MULT,
                    accum_out=acc_b[:, mi:mi + 1])
                if T > 1:
                    # relu(s_1)^2 = max(drb,0)*drb  (s_1 == drb exactly)
                    nc.vector.scalar_tensor_tensor(
                        junkD[:], drb[:, mi, :], 0.0, drb[:, mi, :],
                        op0=MAX, op1=MULT, accum_out=acc_r[:, mi:mi + 1])

            # ---- weight pools: the small matrices coexist with the drive
            # pool (so their softplus fills ACT/DVE idle time during the
            # drive matmuls); W_EE gets the released drive space afterwards.
            theta_sched = []
            if T > 1:
                wps_cm = tc.tile_pool(name="wpool_small", bufs=1)
                wps = wps_cm.__enter__()
                w_ei = wps.tile([128, KI, NE], f16, tag="w_ei")
                w_ie = wps.tile([128, KE, NI], f16, tag="w_ie")
                w_ii = wps.tile([128, KI, NI], f16, tag="w_ii")
                ts_cm = tc.tile_pool(name="tstage", bufs=3)
                ts_ = ts_cm.__enter__()
                # priority order: what step-1's earliest groups need first
                theta_sched += [(thEIT, w_ei, k, 0) for k in range(KI)]
                theta_sched += [(thIIT, w_ii, k, 0) for k in range(KI)]
                theta_sched += [(thEIT, w_ei, k, c) for c in range(1, NE // CSZ)
                                for k in range(KI)]
                theta_sched += [(thIET, w_ie, k, 0) for k in range(KE)]
            else:
                wps_cm = ts_cm = None
            w_ee = None

            # ========= Phase 0: drive matmuls (+ fused step 0) =========
            fuse0 = T > 1 and bench_repeat == 0 and _FUSE0[0]
            with tc.tile_pool(name="dhold", bufs=1) as dh:
                x16 = dh.tile([128, KD, N], f16, tag="x16")
                wxe16 = dh.tile([128, KD, NE], f16, tag="wxe16")
                wxi16 = dh.tile([128, KD, NI], f16, tag="wxi16")

                # per-k-chunk DMAs so the first drive groups can trail the
                # transfer instead of waiting for all of it
                for k in range(KD):
                    nc.sync.dma_start(x16[:, k, :], xT[k * 128:(k + 1) * 128, :])
                    nc.sync.dma_start(wxe16[:, k, :],
                                      wxeT[k * 128:(k + 1) * 128, :])
                    nc.sync.dma_start(wxi16[:, k, :],
                                      wxiT[k * 128:(k + 1) * 128, :])

                # drive_e.T + b_e  -> drb_e (fp16), same for I
                for g in range(ME + MI):
                    is_e = g < ME
                    mi = g if is_e else g - ME
                    wx = wxe16 if is_e else wxi16
                    drb, bcol = (drb_e, be_sb) if is_e else (drb_i, bi_sb)
                    p = psp.tile([128, N], f32, tag="p")
                    for k in range(KD):
                        nc.tensor.matmul(p[:], wx[:, k, mi * 128:(mi + 1) * 128],
                                         x16[:, k, :], start=(k == 0),
                                         stop=(k == KD - 1))
                    nc.scalar.activation(drb[:, mi, :], p[:], AF.Identity,
                                         bias=bcol[:, mi:mi + 1])
                    if fuse0:
                        j0_tile(g, light=True)
                        for _ in range(3):
                            if theta_sched:
                                softplus_chunk(*theta_sched.pop(0))
            # leftover small-matrix chunks (and all of them on the
            # non-fused paths)
            while theta_sched:
                softplus_chunk(*theta_sched.pop(0))

            if T > 1:
                wee_cm = tc.tile_pool(name="wpool_ee", bufs=1)
                wee = wee_cm.__enter__()
                w_ee = wee.tile([128, KE, NE], f16, tag="w_ee")
            else:
                wee_cm = None

            # ================= Phase 2: recurrence ====================
            fo_cell = [None]
            if True:
                def emit_step(j, pre_group=None):
                    rj = r_e[j % 2]        # r_e tiles read this step
                    rn = r_e[(j + 1) % 2]  # r_e tiles written this step
                    last = (j == T - 1)

                    if j == 0:
                        for g in range(ME + MI):
                            j0_tile(g)
                        if last:
                            for mi in range(ME):
                                fo_t = fo_cell[0].tile([128, N], f32, tag="fo")
                                nc.scalar.activation(fo_t[:], s_e[:, mi, :],
                                                     AF.Relu, scale=DT)
                                nc.sync.dma_start(
                                    e_out[mi * 128:(mi + 1) * 128, :], fo_t[:])
                        return

                    # E-side groups: psum = W_EE@r_e + W_EI@r_i
                    for mi in range(ME):
                        if pre_group is not None:
                            pre_group(mi)
                        p = psp.tile([128, N], f32, tag="p")
                        for k in range(KE):
                            nc.tensor.matmul(
                                p[:], w_ee[:, k, mi * 128:(mi + 1) * 128],
                                rj[:, k, :], start=(k == 0), stop=False)
                        for k in range(KI):
                            nc.tensor.matmul(
                                p[:], w_ei[:, k, mi * 128:(mi + 1) * 128],
                                r_i[:, k, :], start=False, stop=(k == KI - 1))
                        # net = rec + drive(+b)   (in-place in PSUM)
                        nc.vector.tensor_tensor(p[:], p[:], drb_e[:, mi, :],
                                                op=ADD)
                        # s' = (1-dt)*s + net     (fused, in-place)
                        nc.vector.scalar_tensor_tensor(
                            s_e[:, mi, :], s_e[:, mi, :], 1.0 - DT, p[:],
                            op0=MULT, op1=ADD)
                        # balance^2 sums; squares scratched into dead psum
                        nc.scalar.activation(
                            p[:], p[:], AF.Square, bias=nbe_sb[:, mi:mi + 1],
                            accum_out=acc_be[:, j * ME + mi:j * ME + mi + 1])
                        if not last:
                            nc.scalar.activation(rn[:, mi, :], s_e[:, mi, :],
                                                 AF.Relu, scale=SE)
                            nc.vector.scalar_tensor_tensor(
                                junkD[:], s_e[:, mi, :], 0.0, s_e[:, mi, :],
                                op0=MAX, op1=MULT,
                                accum_out=acc_re[:, j * ME + mi:
                                                 j * ME + mi + 1])
                        else:
                            fo_t = fo_cell[0].tile([128, N], f32, tag="fo")
                            nc.scalar.activation(fo_t[:], s_e[:, mi, :],
                                                 AF.Relu, scale=DT)
                            nc.sync.dma_start(
                                e_out[mi * 128:(mi + 1) * 128, :], fo_t[:])

                    # I-side groups: psum = W_IE@r_e + W_II@r_i
                    for mi in range(MI):
                        p = psp.tile([128, N], f32, tag="p")
                        for k in range(KE):
                            nc.tensor.matmul(
                                p[:], w_ie[:, k, mi * 128:(mi + 1) * 128],
                                rj[:, k, :], start=(k == 0), stop=False)
                        for k in range(KI):
                            nc.tensor.matmul(
                                p[:], w_ii[:, k, mi * 128:(mi + 1) * 128],
                                r_i[:, k, :], start=False, stop=(k == KI - 1))
                        nc.vector.tensor_tensor(p[:], p[:], drb_i[:, mi, :],
                                                op=ADD)
                        if not last:  # s_i after the last step is never read
                            nc.vector.scalar_tensor_tensor(
                                s_i[:, mi, :], s_i[:, mi, :], 1.0 - DT, p[:],
                                op0=MULT, op1=ADD)
                        nc.scalar.activation(
                            p[:], p[:], AF.Square, bias=nbi_sb[:, mi:mi + 1],
                            accum_out=acc_bi[:, j * MI + mi:j * MI + mi + 1])
                    # r_i is single-buffered: only overwrite it after ALL of
                    # this step's W_II matmuls (which read the old value)
                    # have been emitted. The next step's first EI matmuls
                    # wait on these, so emit them before the F squares.
                    if not last:
                        for mi in range(MI):
                            nc.vector.tensor_scalar(
                                r_i[:, mi, :], s_i[:, mi, :], SI_NEG, 0.0,
                                op0=MULT, op1=MIN)
                        for mi in range(MI):
                            nc.vector.scalar_tensor_tensor(
                                junkD[:], s_i[:, mi, :], 0.0, s_i[:, mi, :],
                                op0=MAX, op1=MULT,
                                accum_out=acc_ri[:, j * MI + mi:
                                                 j * MI + mi + 1])

                EB = NE // CSZ  # number of 512-wide W_EE column blocks (4)
                GPB = ME // EB  # E-groups consuming one block (4)

                def fout_ctx():
                    cm = tc.tile_pool(name="fout", bufs=2)
                    fo_cell[0] = cm.__enter__()
                    return cm

                if T == 1:
                    fcm = fout_ctx()
                    emit_step(0)
                    fcm.__exit__(None, None, None)
                elif bench_repeat:
                    # timing mode: repeat the matmul steps in a HW loop so
                    # device time dominates the per-launch RPC overhead
                    softplus_into(thEET, w_ee, KE, NE)
                    emit_step(0)
                    fcm = fout_ctx()
                    with tc.For_i(0, bench_repeat, 1):
                        for j in range(1, T):
                            emit_step(j)
                    fcm.__exit__(None, None, None)
                else:
                    # step 0 + small-matrix softplus already fused into the
                    # drive phase; W_EE column blocks just-in-time ahead of
                    # the step-1 groups that consume them
                    if not fuse0:
                        emit_step(0)
                    emit_ee_block(0)
                    if T >= 3:
                        def hook(mi):
                            b = mi // GPB + 1
                            if mi % GPB == 0 and b < EB:
                                emit_ee_block(b)
                            # deferred j0 cost sums: 2 tiles per E-group
                            for g in (2 * mi, 2 * mi + 1):
                                if g < ME + MI:
                                    j0_squares(g)
                        emit_step(1, pre_group=hook)
                        fcm = fout_ctx()
                        for j in range(2, T):
                            if j == T - 1:
                                # r^2 accums are complete after step T-2;
                                # reduce them in the shadow of the last step
                                AXx = mybir.AxisListType.X
                                nc.vector.reduce_sum(acc4[:, 0:1], acc_re[:],
                                                     axis=AXx)
                                nc.vector.reduce_sum(acc4[:, 1:2], acc_ri[:],
                                                     axis=AXx)
                            emit_step(j)
                        fcm.__exit__(None, None, None)
                    else:  # T == 2
                        for b in range(1, EB):
                            emit_ee_block(b)
                        fcm = fout_ctx()
                        emit_step(1)
                        fcm.__exit__(None, None, None)

            # ---- final scalar reduction: 4 partial sums ----
            AX = mybir.AxisListType.X
            if not (bench_repeat == 0 and T >= 3):
                nc.vector.reduce_sum(acc4[:, 0:1], acc_re[:], axis=AX)
                nc.vector.reduce_sum(acc4[:, 1:2], acc_ri[:], axis=AX)
            nc.vector.reduce_sum(acc4[:, 2:3], acc_be[:], axis=AX)
            nc.vector.reduce_sum(acc4[:, 3:4], acc_bi[:], axis=AX)
            ones = nc.const_aps.tensor(1.0, (128, 1), f32)
            # shares the "pv" bank (pv is prologue-only, this is end-only)
            ps4 = psp.tile([4, 1], f32, tag="pv", bufs=1, name="ps4")
            nc.tensor.matmul(ps4[:], acc4[:, 0:4], ones, start=True, stop=True)
            nc.vector.tensor_copy(sb4[:], ps4[:])
            nc.sync.dma_start(sums_out[:], sb4[:])
            if wee_cm is not None:
                wee_cm.__exit__(None, None, None)
            if ts_cm is not None:
                ts_cm.__exit__(None, None, None)
            if wps_cm is not None:
                wps_cm.__exit__(None, None, None)

    nc.finalize()
    return nc


def _get_nc(T: int):
    nc = _CACHE.get(T)
    if nc is None:
        nc = _build_nc(T)
        _CACHE[T] = nc
    return nc


def make_in_maps(inputs: dict):
    g = {k: np.ascontiguousarray(np.asarray(v, dtype=np.float32))
         for k, v in inputs.items() if k != "T"}
    f16 = np.float16
    shared = {
        "thEET": np.ascontiguousarray(g["theta_EE"].T).astype(f16),
        "thEIT": np.ascontiguousarray(g["theta_EI"].T).astype(f16),
        "thIET": np.ascontiguousarray(g["theta_IE"].T).astype(f16),
        "thIIT": np.ascontiguousarray(g["theta_II"].T).astype(f16),
        "wxeT": np.ascontiguousarray(g["W_XE_w"].T).astype(f16),
        "wxiT": np.ascontiguousarray(g["W_XI_w"].T).astype(f16),
        # drive_e + b_e is constant across steps: fold both biases into drb
        "be": (g["W_XE_b"] + g["b_e"]).astype(np.float32),
        "bi": (g["W_XI_b"] + g["b_i"]).astype(np.float32),
        # balance = net - b_e: subtract the net-bias alone when squaring
        "bne": g["b_e"],
        "bni": g["b_i"],
    }
    in_maps = []
    for c in range(NCORES):
        m = dict(shared)
        m["xT"] = np.ascontiguousarray(g["x"][c * BS:(c + 1) * BS].T).astype(f16)
        in_maps.append(m)
    return in_maps


def run(inputs: dict, trace: bool = False):
    """Run on 8 cores; returns (outputs_tuple, BassKernelResults)."""
    from concourse import bass_utils

    T = int(np.asarray(inputs["T"]))
    in_maps = make_in_maps(inputs)

    nc = _get_nc(T)
    kwargs = {}
    if trace:
        kwargs = dict(trace=True, trace_cores=[0])
    res = bass_utils.run_bass_kernel_spmd(nc, in_maps,
                                          core_ids=list(range(NCORES)),
                                          **kwargs)

    e_full = np.concatenate([res.results[c]["e_out"].T for c in range(NCORES)],
                            axis=0)
    s = np.zeros(4, dtype=np.float64)
    for c in range(NCORES):
        s += res.results[c]["sums"].astype(np.float64).ravel()
    rsqE, rsqI, bsqE, bsqI = s
    ac = (RSQE_FIX * rsqE / (B * NE) + RSQI_FIX * rsqI / (B * NI)) / T
    bc = (bsqE / (B * NE) + bsqI / (B * NI)) / T
    out = (np.ascontiguousarray(e_full, dtype=np.float32),
           np.float32(ac), np.float32(bc))
    return out, res


def kernel(**inputs):
    out, _ = run(inputs)
    return out


# revision 71
# speedup vs baseline: 247650.5130x; 1.0064x over previous
"""Balanced E/I recurrent layer on 8 Trainium2 NeuronCores.

Contract: kernel(**inputs) takes the FULL inputs (as in reference.setup_inputs)
and returns the FULL output tuple (relu(e_T) [4096,2048] f32, ac/T, bc/T).

Strategy (data-parallel, batch 4096 -> 512 per core, SPMD on 8 cores):
  - All tensors on device are feature-major ([feature, batch_shard]); each
    recurrence step is 20 PSUM accumulation groups of 20 matmuls (N=512).
  - State kept scaled: s = e / dt  => update s' = (1-dt)*s + net is ONE fused
    DVE op (scalar_tensor_tensor); net formed in-place in PSUM.
  - Matmul operands in fp16 (PE truncates to ~fp22 internally anyway; fp16
    runs at 1 cycle/row vs 4 for fp32), accumulation in fp32 PSUM.
  - Weights stored as raw softplus(theta) fp16; the Dale fan-in scale and the
    I-presynaptic sign live on the activations (r_e = relu(e)/NE via ACT,
    r_i = -relu(i)/NI via a DVE mult+min). The tiny r values dip into fp16
    subnormals, which the PE handles exactly (verified on silicon).
  - softplus on device two ways, load-balanced across engines: ACT path
    Ln(Exp(x)+1), DVE path a quadratic fit (err 25x below the fp16 floor);
    W_EE column blocks are produced just-in-time ahead of the step-1 groups
    consuming them, and the small matrices + step 0 are fused into the
    drive-matmul phase so no engine sits behind another's queue.
  - act/bal cost sums: per-(tile,step) partition-column sums via accum_out,
    reduced on-device to 4 scalars per core (ones-matmul over partitions);
    the host sums across cores and applies the scale normalizations.
Measured (For_i repeat-slope on silicon): 4-step recurrence block ~405 us
(~253 ns/matmul vs a pure back-to-back matmul floor of 246.7 ns/matmul on
this silicon, i.e. ~97% of the achievable PE rate).
"""

import numpy as np

B, D, NE, NI = 4096, 1024, 2048, 512
NCORES = 8
BS = B // NCORES          # 512 batch rows per core
N = BS                    # moving (free) dim of all matmuls
KE, KI, KD = NE // 128, NI // 128, D // 128   # 16, 4, 8 contraction tiles
ME, MI = NE // 128, NI // 128                 # 16, 4 output tiles
DT = 0.2
# Weights stored as raw softplus(theta) in fp16 (positive); the full Dale
# fan-in scale and the I-side sign live on the activations:
#   r_e16 = relu(e)/NE = Relu(s_e * DT/NE)      (ACT)
#   r_i16 = -relu(i)/NI = min(s_i * -DT/NI, 0)  (DVE)
# The tiny r16 values dip into fp16 subnormals; verified exact on the PE.
SE = DT / NE
SI_NEG = -DT / NI
# device accumulates sum(relu(s)^2) with s = e/dt  =>  relu(e)^2 = dt^2 * ...
RSQE_FIX = DT * DT
RSQI_FIX = DT * DT

_CACHE: dict[int, object] = {}
_FUSE0 = [True]  # fuse step-0 + small softplus into the drive phase


def _build_nc(T: int, bench_repeat: int = 0):
    import concourse.mybir as mybir
    import concourse.tile as tile
    from concourse import bacc

    AF = mybir.ActivationFunctionType
    f32, f16 = mybir.dt.float32, mybir.dt.float16
    ADD, MULT, MAX, MIN, BYPASS = (
        mybir.AluOpType.add, mybir.AluOpType.mult, mybir.AluOpType.max,
        mybir.AluOpType.min, mybir.AluOpType.bypass)

    # All ACT funcs used here (Exp/Ln/Identity/Relu/Square/Copy) coexist in
    # the 'natural_log_exp_and_others' table. The stock table-load pass picks
    # first-fit per func, thrashing tables (~156 loads x 1.3us). Restrict the
    # choice to the one covering table -> a single load.
    class _Bacc(bacc.Bacc):
        def insert_act_table_loads(self):
            from concourse.hw_specs import get_activation_tables
            import bass_rust as _bass_rust
            has_activation = any(
                isinstance(i, mybir.InstActivation)
                for b in self.main_func.blocks
                for i in b.instructions
            )
            if not has_activation:
                return
            tables = list(get_activation_tables(self.m.arch).items())
            full = "natural_log_exp_and_others"
            if any(n == full for n, _ in tables):
                tables = [(n, (s if n == full else set())) for n, s in tables]
            _bass_rust.insert_act_table_loads(self, tables)

    nc = _Bacc("TRN2", target_bir_lowering=False, debug=False,
               num_devices=NCORES, name="bei")

    # ---- I/O ----
    # matmul operands arrive as fp16 (host-side cast: identical rounding to
    # the on-device convert it replaces, at half the DMA bytes)
    xT = nc.dram_tensor("xT", (D, BS), f16, kind="ExternalInput").ap()
    thEET = nc.dram_tensor("thEET", (NE, NE), f16, kind="ExternalInput").ap()
    thEIT = nc.dram_tensor("thEIT", (NI, NE), f16, kind="ExternalInput").ap()
    thIET = nc.dram_tensor("thIET", (NE, NI), f16, kind="ExternalInput").ap()
    thIIT = nc.dram_tensor("thIIT", (NI, NI), f16, kind="ExternalInput").ap()
    wxeT = nc.dram_tensor("wxeT", (D, NE), f16, kind="ExternalInput").ap()
    wxiT = nc.dram_tensor("wxiT", (D, NI), f16, kind="ExternalInput").ap()
    be = nc.dram_tensor("be", (NE,), f32, kind="ExternalInput").ap()
    bi = nc.dram_tensor("bi", (NI,), f32, kind="ExternalInput").ap()
    # net-bias alone (b_e/b_i), subtracted when squaring balance = net - b
    bne = nc.dram_tensor("bne", (NE,), f32, kind="ExternalInput").ap()
    bni = nc.dram_tensor("bni", (NI,), f32, kind="ExternalInput").ap()

    e_out = nc.dram_tensor("e_out", (NE, BS), f32, kind="ExternalOutput").ap()
    sums_out = nc.dram_tensor("sums", (4, 1), f32, kind="ExternalOutput").ap()

    n_r_steps = max(T - 1, 1)  # steps contributing r^2 (j = 0..T-2)

    with tile.TileContext(nc) as tc:
        with (
            tc.tile_pool(name="persist", bufs=1) as pp,
            tc.tile_pool(name="psum", bufs=5, space="PSUM") as psp,
        ):
            # ---- persistent state ----
            s_e = pp.tile([128, KE, N], f32, tag="s_e")
            s_i = pp.tile([128, KI, N], f32, tag="s_i")
            r_e = [pp.tile([128, KE, N], f16, tag="r_e0", name="r_e0"),
                   pp.tile([128, KE, N], f16, tag="r_e1", name="r_e1")]
            r_i = pp.tile([128, KI, N], f16, tag="r_i")
            drb_e = pp.tile([128, ME, N], f16, tag="drb_e")
            drb_i = pp.tile([128, MI, N], f16, tag="drb_i")
            acc_re = pp.tile([128, n_r_steps * ME], f32, tag="acc_re")
            acc_ri = pp.tile([128, n_r_steps * MI], f32, tag="acc_ri")
            acc_be = pp.tile([128, T * ME], f32, tag="acc_be")
            acc_bi = pp.tile([128, T * MI], f32, tag="acc_bi")
            acc4 = pp.tile([128, 4], f32, tag="acc4")
            junkD = pp.tile([128, N], f32, tag="junkD")  # DVE-only discard
            sb4 = pp.tile([4, 1], f32, tag="sb4")
            be_sb = pp.tile([128, ME], f32, tag="be_sb")
            bi_sb = pp.tile([128, MI], f32, tag="bi_sb")
            nbe_sb = pp.tile([128, ME], f32, tag="nbe_sb")
            nbi_sb = pp.tile([128, MI], f32, tag="nbi_sb")

            nc.gpsimd.memset(acc_re[:], 0.0)
            nc.gpsimd.memset(acc_ri[:], 0.0)
            nc.gpsimd.memset(acc_be[:], 0.0)
            nc.gpsimd.memset(acc_bi[:], 0.0)

            nc.sync.dma_start(be_sb[:], be.rearrange("(t p) -> p t", p=128))
            nc.sync.dma_start(bi_sb[:], bi.rearrange("(t p) -> p t", p=128))
            nc.sync.dma_start(nbe_sb[:], bne.rearrange("(t p) -> p t", p=128))
            nc.sync.dma_start(nbi_sb[:], bni.rearrange("(t p) -> p t", p=128))
            nc.vector.tensor_scalar_mul(nbe_sb[:], nbe_sb[:], -1.0)
            nc.vector.tensor_scalar_mul(nbi_sb[:], nbi_sb[:], -1.0)

            # W = softplus(theta) (raw, positive, fp16), two ways split
            # across ACT and DVE:
            #   ACT path: Ln(Exp(x) + 1) -> fp16 directly
            #   DVE path: quadratic minimax fit on [-0.45, 0.45]
            #     (max err 4.8e-5, ~25x below the fp16 rounding floor)
            C0, C1, C2 = 0.69316522, 0.5, 0.12410602
            chunk_ctr = [0]
            CSZ = 512

            def softplus_chunk(theta_ap, w_tile, k, c, force_act=False):
                st = ts_.tile([128, CSZ], f16, tag="tchunk", name="tchunk")
                nc.sync.dma_start(
                    st[:], theta_ap[k * 128:(k + 1) * 128,
                                    c * CSZ:(c + 1) * CSZ])
                w_out = w_tile[:, k, c * CSZ:(c + 1) * CSZ]
                i = chunk_ctr[0]
                chunk_ctr[0] += 1
                if (i % 8 < 3) and not force_act:
                    # DVE polynomial path
                    v0 = psp.tile([128, CSZ], f32, tag="pv", bufs=1,
                                  name="pv")
                    nc.vector.tensor_scalar(v0[:], st[:], C1, C0,
                                            op0=MULT, op1=ADD)
                    nc.vector.tensor_tensor(st[:], st[:], st[:], op=MULT)
                    nc.vector.scalar_tensor_tensor(w_out, st[:], C2, v0[:],
                                                   op0=MULT, op1=ADD)
                else:
                    # Exp to f32 psum scratch (avoids an extra fp16
                    # rounding between Exp and Ln)
                    ex = psp.tile([128, CSZ], f32, tag="pex", bufs=2,
                                  name="pex")
                    nc.scalar.activation(ex[:], st[:], AF.Exp)
                    nc.scalar.activation(w_out, ex[:], AF.Ln, bias=1.0)

            def softplus_into(theta_ap, w_tile, kt, width):
                for k in range(kt):
                    for c in range(width // CSZ):
                        softplus_chunk(theta_ap, w_tile, k, c)

            def emit_ee_block(b, force_act=False):
                # one 512-wide column block of W_EE.T = what E-groups
                # 4b..4b+3 of the next step consume
                for k in range(KE):
                    softplus_chunk(thEET, w_ee, k, b, force_act=force_act)

            # step-0 for one tile g (E tiles 0..ME-1, then I tiles):
            # s_1 = net_0 = drive(+bias); no matmuls since e_0 = 0
            def j0_tile(g, light=False):
                is_e = g < ME
                mi = g if is_e else g - ME
                drb, nb = (drb_e, nbe_sb) if is_e else (drb_i, nbi_sb)
                s_t = s_e if is_e else s_i
                nc.vector.tensor_copy(s_t[:, mi, :], drb[:, mi, :])
                if T > 1:
                    if is_e:
                        # on DVE (not ACT) to keep the drive window balanced:
                        # (s*SE) max 0 == relu(s*SE) bit-for-bit
                        nc.vector.tensor_scalar(r_e[1][:, mi, :],
                                                s_t[:, mi, :], SE, 0.0,
                                                op0=MULT, op1=MAX)
                    else:
                        nc.vector.tensor_scalar(r_i[:, mi, :], s_t[:, mi, :],
                                                SI_NEG, 0.0, op0=MULT,
                                                op1=MIN)
                if not light:
                    j0_squares(g)

            # j0's cost sums read only drb (constant), so they can run as
            # background DVE work any time later (deferred into step 1)
            def j0_squares(g):
                is_e = g < ME
                mi = g if is_e else g - ME
                drb, nb = (drb_e, nbe_sb) if is_e else (drb_i, nbi_sb)
                acc_b, acc_r = (acc_be, acc_re) if is_e else (acc_bi, acc_ri)
                # balance_0^2 = (drb - b)^2: u = drb - b, then u*u with accum
                nc.vector.tensor_scalar(junkD[:], drb[:, mi, :],
                                        nb[:, mi:mi + 1], None, op0=ADD)
                nc.vector.scalar_tensor_tensor(
                    junkD[:], junkD[:], 0.0, junkD[:], op0=BYPASS, op1=MULT,
                    accum_out=acc_b[:, mi:mi + 1])
                if T > 1:
                    # relu(s_1)^2 = max(drb,0)*drb  (s_1 == drb exactly)
                    nc.vector.scalar_tensor_tensor(
                        junkD[:], drb[:, mi, :], 0.0, drb[:, mi, :],
                        op0=MAX, op1=MULT, accum_out=acc_r[:, mi:mi + 1])

            # ---- weight pools: the small matrices coexist with the drive
            # pool (so their softplus fills ACT/DVE idle time during the
            # drive matmuls); W_EE gets the released drive space afterwards.
            theta_sched = []
            if T > 1:
                wps_cm = tc.tile_pool(name="wpool_small", bufs=1)
                wps = wps_cm.__enter__()
                w_ei = wps.tile([128, KI, NE], f16, tag="w_ei")
                w_ie = wps.tile([128, KE, NI], f16, tag="w_ie")
                w_ii = wps.tile([128, KI, NI], f16, tag="w_ii")
                ts_cm = tc.tile_pool(name="tstage", bufs=3)
                ts_ = ts_cm.__enter__()
                # priority order: what step-1's earliest groups need first
                theta_sched += [(thEIT, w_ei, k, 0) for k in range(KI)]
                theta_sched += [(thIIT, w_ii, k, 0) for k in range(KI)]
                theta_sched += [(thEIT, w_ei, k, c) for c in range(1, NE // CSZ)
                                for k in range(KI)]
                theta_sched += [(thIET, w_ie, k, 0) for k in range(KE)]
            else:
                wps_cm = ts_cm = None
            w_ee = None

            # ========= Phase 0: drive matmuls (+ fused step 0) =========
            fuse0 = T > 1 and bench_repeat == 0 and _FUSE0[0]
            with tc.tile_pool(name="dhold", bufs=1) as dh:
                x16 = dh.tile([128, KD, N], f16, tag="x16")
                wxe16 = dh.tile([128, KD, NE], f16, tag="wxe16")
                wxi16 = dh.tile([128, KD, NI], f16, tag="wxi16")

                # per-k-chunk DMAs so the first drive groups can trail the
                # transfer instead of waiting for all of it
                for k in range(KD):
                    nc.sync.dma_start(x16[:, k, :], xT[k * 128:(k + 1) * 128, :])
                    nc.sync.dma_start(wxe16[:, k, :],
                                      wxeT[k * 128:(k + 1) * 128, :])
                    nc.sync.dma_start(wxi16[:, k, :],
                                      wxiT[k * 128:(k + 1) * 128, :])

                # drive_e.T + b_e  -> drb_e (fp16), same for I
                for g in range(ME + MI):
                    is_e = g < ME
                    mi = g if is_e else g - ME
                    wx = wxe16 if is_e else wxi16
                    drb, bcol = (drb_e, be_sb) if is_e else (drb_i, bi_sb)
                    p = psp.tile([128, N], f32, tag="p")
                    for k in range(KD):
                        nc.tensor.matmul(p[:], wx[:, k, mi * 128:(mi + 1) * 128],
                                         x16[:, k, :], start=(k == 0),
                                         stop=(k == KD - 1))
                    nc.scalar.activation(drb[:, mi, :], p[:], AF.Identity,
                                         bias=bcol[:, mi:mi + 1])
                    if fuse0:
                        j0_tile(g, light=True)
                        for _ in range(3):
                            if theta_sched:
                                softplus_chunk(*theta_sched.pop(0))
            # leftover small-matrix chunks (and all of them on the
            # non-fused paths); ACT path -- post-drive, DVE is busy with
            # step 0 while ACT is otherwise idle
            while theta_sched:
                softplus_chunk(*theta_sched.pop(0), force_act=fuse0)

            if T > 1:
                wee_cm = tc.tile_pool(name="wpool_ee", bufs=1)
                wee = wee_cm.__enter__()
                w_ee = wee.tile([128, KE, NE], f16, tag="w_ee")
            else:
                wee_cm = None

            # ================= Phase 2: recurrence ====================
            fo_cell = [None]
            if True:
                def emit_step(j, pre_group=None):
                    rj = r_e[j % 2]        # r_e tiles read this step
                    rn = r_e[(j + 1) % 2]  # r_e tiles written this step
                    last = (j == T - 1)

                    if j == 0:
                        for g in range(ME + MI):
                            j0_tile(g)
                        if last:
                            for mi in range(ME):
                                fo_t = fo_cell[0].tile([128, N], f32, tag="fo")
                                nc.scalar.activation(fo_t[:], s_e[:, mi, :],
                                                     AF.Relu, scale=DT)
                                nc.sync.dma_start(
                                    e_out[mi * 128:(mi + 1) * 128, :], fo_t[:])
                        return

                    # E-side groups: psum = W_EE@r_e + W_EI@r_i
                    for mi in range(ME):
                        if pre_group is not None:
                            pre_group(mi)
                        p = psp.tile([128, N], f32, tag="p")
                        for k in range(KE):
                            nc.tensor.matmul(
                                p[:], w_ee[:, k, mi * 128:(mi + 1) * 128],
                                rj[:, k, :], start=(k == 0), stop=False)
                        for k in range(KI):
                            nc.tensor.matmul(
                                p[:], w_ei[:, k, mi * 128:(mi + 1) * 128],
                                r_i[:, k, :], start=False, stop=(k == KI - 1))
                        # net = rec + drive(+b)   (in-place in PSUM)
                        nc.vector.tensor_tensor(p[:], p[:], drb_e[:, mi, :],
                                                op=ADD)
                        # s' = (1-dt)*s + net     (fused, in-place)
                        nc.vector.scalar_tensor_tensor(
                            s_e[:, mi, :], s_e[:, mi, :], 1.0 - DT, p[:],
                            op0=MULT, op1=ADD)
                        # balance^2 sums; squares scratched into dead psum
                        nc.scalar.activation(
                            p[:], p[:], AF.Square, bias=nbe_sb[:, mi:mi + 1],
                            accum_out=acc_be[:, j * ME + mi:j * ME + mi + 1])
                        if not last:
                            nc.scalar.activation(rn[:, mi, :], s_e[:, mi, :],
                                                 AF.Relu, scale=SE)
                            nc.vector.scalar_tensor_tensor(
                                junkD[:], s_e[:, mi, :], 0.0, s_e[:, mi, :],
                                op0=MAX, op1=MULT,
                                accum_out=acc_re[:, j * ME + mi:
                                                 j * ME + mi + 1])
                        else:
                            fo_t = fo_cell[0].tile([128, N], f32, tag="fo")
                            nc.scalar.activation(fo_t[:], s_e[:, mi, :],
                                                 AF.Relu, scale=DT)
                            nc.sync.dma_start(
                                e_out[mi * 128:(mi + 1) * 128, :], fo_t[:])

                    # I-side groups: psum = W_IE@r_e + W_II@r_i
                    for mi in range(MI):
                        p = psp.tile([128, N], f32, tag="p")
                        for k in range(KE):
                            nc.tensor.matmul(
                                p[:], w_ie[:, k, mi * 128:(mi + 1) * 128],
                                rj[:, k, :], start=(k == 0), stop=False)
                        for k in range(KI):
                            nc.tensor.matmul(
                                p[:], w_ii[:, k, mi * 128:(mi + 1) * 128],
                                r_i[:, k, :], start=False, stop=(k == KI - 1))
                        nc.vector.tensor_tensor(p[:], p[:], drb_i[:, mi, :],
                                                op=ADD)
                        if not last:  # s_i after the last step is never read
                            nc.vector.scalar_tensor_tensor(
                                s_i[:, mi, :], s_i[:, mi, :], 1.0 - DT, p[:],
                                op0=MULT, op1=ADD)
                        nc.scalar.activation(
                            p[:], p[:], AF.Square, bias=nbi_sb[:, mi:mi + 1],
                            accum_out=acc_bi[:, j * MI + mi:j * MI + mi + 1])
                    # r_i is single-buffered: only overwrite it after ALL of
                    # this step's W_II matmuls (which read the old value)
                    # have been emitted. The next step's first EI matmuls
                    # wait on these, so emit them before the F squares.
                    if not last:
                        for mi in range(MI):
                            nc.vector.tensor_scalar(
                                r_i[:, mi, :], s_i[:, mi, :], SI_NEG, 0.0,
                                op0=MULT, op1=MIN)
                        for mi in range(MI):
                            nc.vector.scalar_tensor_tensor(
                                junkD[:], s_i[:, mi, :], 0.0, s_i[:, mi, :],
                                op0=MAX, op1=MULT,
                                accum_out=acc_ri[:, j * MI + mi:
                                                 j * MI + mi + 1])

                EB = NE // CSZ  # number of 512-wide W_EE column blocks (4)
                GPB = ME // EB  # E-groups consuming one block (4)

                def fout_ctx():
                    cm = tc.tile_pool(name="fout", bufs=2)
                    fo_cell[0] = cm.__enter__()
                    return cm

                if T == 1:
                    fcm = fout_ctx()
                    emit_step(0)
                    fcm.__exit__(None, None, None)
                elif bench_repeat:
                    # timing mode: repeat the matmul steps in a HW loop so
                    # device time dominates the per-launch RPC overhead
                    softplus_into(thEET, w_ee, KE, NE)
                    emit_step(0)
                    fcm = fout_ctx()
                    with tc.For_i(0, bench_repeat, 1):
                        for j in range(1, T):
                            emit_step(j)
                    fcm.__exit__(None, None, None)
                else:
                    # step 0 + small-matrix softplus already fused into the
                    # drive phase; W_EE column blocks just-in-time ahead of
                    # the step-1 groups that consume them
                    if not fuse0:
                        emit_step(0)
                    emit_ee_block(0, force_act=fuse0)
                    if T >= 3:
                        def hook(mi):
                            b = mi // GPB + 1
                            if mi % GPB == 0 and b < EB:
                                emit_ee_block(b)
                            # deferred j0 cost sums: 2 tiles per E-group
                            for g in (2 * mi, 2 * mi + 1):
                                if g < ME + MI:
                                    j0_squares(g)
                        emit_step(1, pre_group=hook)
                        fcm = fout_ctx()
                        for j in range(2, T):
                            if j == T - 1:
                                # r^2 accums are complete after step T-2;
                                # reduce them in the shadow of the last step
                                AXx = mybir.AxisListType.X
                                nc.vector.reduce_sum(acc4[:, 0:1], acc_re[:],
                                                     axis=AXx)
                                nc.vector.reduce_sum(acc4[:, 1:2], acc_ri[:],
                                                     axis=AXx)
                            emit_step(j)
                        fcm.__exit__(None, None, None)
                    else:  # T == 2
                        for b in range(1, EB):
                            emit_ee_block(b)
                        fcm = fout_ctx()
                        emit_step(1)
                        fcm.__exit__(None, None, None)

            # ---- final scalar reduction: 4 partial sums ----
            AX = mybir.AxisListType.X
            if not (bench_repeat == 0 and T >= 3):
                nc.vector.reduce_sum(acc4[:, 0:1], acc_re[:], axis=AX)
                nc.vector.reduce_sum(acc4[:, 1:2], acc_ri[:], axis=AX)
            nc.vector.reduce_sum(acc4[:, 2:3], acc_be[:], axis=AX)
            nc.vector.reduce_sum(acc4[:, 3:4], acc_bi[:], axis=AX)
            ones = nc.const_aps.tensor(1.0, (128, 1), f32)
            # shares the "pv" bank (pv is prologue-only, this is end-only)
            ps4 = psp.tile([4, 1], f32, tag="pv", bufs=1, name="ps4")
            nc.tensor.matmul(ps4[:], acc4[:, 0:4], ones, start=True, stop=True)
            nc.vector.tensor_copy(sb4[:], ps4[:])
            nc.sync.dma_start(sums_out[:], sb4[:])
            if wee_cm is not None:
                wee_cm.__exit__(None, None, None)
            if ts_cm is not None:
                ts_cm.__exit__(None, None, None)
            if wps_cm is not None:
                wps_cm.__exit__(None, None, None)

    nc.finalize()
    return nc


def _get_nc(T: int):
    nc = _CACHE.get(T)
    if nc is None:
        nc = _build_nc(T)
        _CACHE[T] = nc
    return nc


def make_in_maps(inputs: dict):
    g = {k: np.ascontiguousarray(np.asarray(v, dtype=np.float32))
         for k, v in inputs.items() if k != "T"}
    f16 = np.float16
    shared = {
        "thEET": np.ascontiguousarray(g["theta_EE"].T).astype(f16),
        "thEIT": np.ascontiguousarray(g["theta_EI"].T).astype(f16),
        "thIET": np.ascontiguousarray(g["theta_IE"].T).astype(f16),
        "thIIT": np.ascontiguousarray(g["theta_II"].T).astype(f16),
        "wxeT": np.ascontiguousarray(g["W_XE_w"].T).astype(f16),
        "wxiT": np.ascontiguousarray(g["W_XI_w"].T).astype(f16),
        # drive_e + b_e is constant across steps: fold both biases into drb
        "be": (g["W_XE_b"] + g["b_e"]).astype(np.float32),
        "bi": (g["W_XI_b"] + g["b_i"]).astype(np.float32),
        # balance = net - b_e: subtract the net-bias alone when squaring
        "bne": g["b_e"],
        "bni": g["b_i"],
    }
    in_maps = []
    for c in range(NCORES):
        m = dict(shared)
        m["xT"] = np.ascontiguousarray(g["x"][c * BS:(c + 1) * BS].T).astype(f16)
        in_maps.append(m)
    return in_maps


def run(inputs: dict, trace: bool = False):
    """Run on 8 cores; returns (outputs_tuple, BassKernelResults)."""
    from concourse import bass_utils

    T = int(np.asarray(inputs["T"]))
    in_maps = make_in_maps(inputs)

    nc = _get_nc(T)
    kwargs = {}
    if trace:
        kwargs = dict(trace=True, trace_cores=[0])
    res = bass_utils.run_bass_kernel_spmd(nc, in_maps,
                                          core_ids=list(range(NCORES)),
                                          **kwargs)

    e_full = np.concatenate([res.results[c]["e_out"].T for c in range(NCORES)],
                            axis=0)
    s = np.zeros(4, dtype=np.float64)
    for c in range(NCORES):
        s += res.results[c]["sums"].astype(np.float64).ravel()
    rsqE, rsqI, bsqE, bsqI = s
    ac = (RSQE_FIX * rsqE / (B * NE) + RSQI_FIX * rsqI / (B * NI)) / T
    bc = (bsqE / (B * NE) + bsqI / (B * NI)) / T
    out = (np.ascontiguousarray(e_full, dtype=np.float32),
           np.float32(ac), np.float32(bc))
    return out, res


def kernel(**inputs):
    out, _ = run(inputs)
    return out
